# revision 1
# baseline (speedup 1.0000x reference)
"""BiLSTM-CRF loss kernel for Trainium2 (8 NeuronCores, data-parallel over batch).

Strategy:
  - Each of the 8 cores processes 8 of the 64 sequences end-to-end (embedding
    gather, BiLSTM, emissions, CRF numerator + forward algorithm). No
    collectives; the host sums the 64 per-sequence scores into the scalar loss.
  - LSTM matmuls run in bf16 (f32 PSUM accumulation); CRF runs in f32.
  - Layout: z^T = W @ h form, i.e. gates on partitions ([128, 8 m-tiles, 8
    batch] per step), so gate bias folds into the precomputed input
    projection and the whole gate nonlinearity pass is ~9 wide ops per step.
"""

import os
import numpy as np
import ml_dtypes

import concourse.bass as bass
import concourse.tile as tile
from concourse import mybir
from concourse.bass_utils import run_bass_kernel_spmd
from concourse.masks import make_identity
from contextlib import ExitStack

# ---------------------------------------------------------------------------
# Workaround: this compiler build allows at most 2 sem waits on a CTRL (Drain)
# instruction; TileContext's tail drain can carry more. Split the waits across
# chained drains on the same engine.
from concourse import tile as _tile_mod
from concourse.vector_clock import ScopedClock as _ScopedClock

_MAX_DRAIN_WAITS = 1


def _split_drain_and_barrier(self, tick_clock, wait_clock):
    nc = self.nc
    drain_inst = nc.sync.drain()
    wait_clock.add_sem_waits(
        drain_inst.ins, _ScopedClock({None: tick_clock.global_clock})
    )
    si = drain_inst.ins.sync_info
    waits = list(si.on_wait or []) if si is not None else []
    if len(waits) > _MAX_DRAIN_WAITS:
        si.on_wait = waits[:_MAX_DRAIN_WAITS]
        for i in range(_MAX_DRAIN_WAITS, len(waits), _MAX_DRAIN_WAITS):
            d = nc.sync.drain()
            dsi = d.ins.sync_info
            if dsi is None:
                d.ins.sync_info = si
                dsi = d.ins.sync_info
            dsi.on_wait = waits[i : i + _MAX_DRAIN_WAITS]
            dsi.on_update = []
    nc.all_engine_barrier()
    assert self.sems is not None
    popped = nc._tile_sem_poison_stack.pop()
    assert popped is self._sem_poison
    nc.clear_and_free_semaphores(list(self.sems.allocated().values()))
    nc.all_engine_barrier()


_tile_mod.TileContext._drain_and_barrier = _split_drain_and_barrier


def _fixup_wait_limit(nc, max_waits=1):
    """This compiler build supports at most 2 sem waits per TPB instruction.
    Split excess waits onto same-engine NOPs inserted right before the
    offending instruction (safe: a nop has no updates, so nothing else is
    delayed beyond what the original multi-wait stall already implied)."""
    main_insts = nc.cur_bb.bb.instructions

    def make_nop(engine):
        eng = nc.engines[engine]
        bi = eng.drain(fusable=False)
        nop = bi.ins
        assert main_insts[-1].name == nop.name
        main_insts.pop()
        return nop

    from concourse import mybir as _mybir

    for f in nc.m.functions:
        for bb in f.blocks:
            insts = bb.instructions
            idx = 0
            while idx < len(insts):
                inst = insts[idx]
                si = inst.sync_info
                lim = max_waits
                waits = list(si.on_wait) if (si is not None and si.on_wait) else []
                if len(waits) > lim:
                    si.on_wait = waits[:lim]
                    excess = waits[lim:]
                    for j in range(0, len(excess), 1):
                        nop = make_nop(inst.engine)
                        nop.sync_info = _mybir.SyncInfo(
                            on_wait=excess[j : j + 1], on_update=[]
                        )
                        insts.insert(idx, nop)
                        idx += 1
                idx += 1


# ---------------------------------------------------------------------------

VOCAB = 50000
TAGSET = 10
NT = TAGSET - 1  # 9 CRF tags
E = 256
HID = 512
Hd = HID // 2  # 256 per direction
B = 64
S_FULL = 256
NCORES = 8
Bc = B // NCORES  # 8 sequences per core

BF16 = mybir.dt.bfloat16
F32 = mybir.dt.float32
I32 = mybir.dt.int32
AF = mybir.ActivationFunctionType
ALU = mybir.AluOpType
nbf16 = ml_dtypes.bfloat16


def _bcast_mid(ap, n):
    """AP [p, m] -> [p, n(bcast), m]"""
    return bass.AP(tensor=ap.tensor, offset=ap.offset, ap=[ap.ap[0], [0, n], ap.ap[1]])


def build_program(S, repeat=1):
    """Build the SPMD Bass program for sequence length S (S % 16 == 0)."""
    TOK = S * Bc            # tokens per core, ordered tau = t*Bc + b
    NTILE = TOK // 128      # 128-token tiles
    TPT = 128 // Bc         # timesteps per token tile (16)
    CW = min(512, TOK)      # inproj psum chunk width

    nc = bass.Bass()

    def din(name, shape, dt):
        return nc.dram_tensor(name, shape, dt, kind="ExternalInput")

    ids_d = din("ids", [128, NTILE], I32)
    emb_d = din("emb", [VOCAB, E], F32)
    wih_d = {d: din(f"wih_{d}", [128, 2, 4 * Hd], BF16) for d in "fb"}
    whh_d = {d: din(f"whh_{d}", [128, 2, 4 * Hd], BF16) for d in "fb"}
    bias_d = {d: din(f"bias_{d}", [128, 8], F32) for d in "fb"}
    wout_d = din("wout", [128, 4, TAGSET], BF16)
    boutr_d = din("boutr", [128, TAGSET], F32)
    trepT_d = din("trepT", [128, NT * NT], F32)   # trans.T flat, replicated
    trepPN_d = din("trepPN", [128, NT * NT], F32)  # trans flat, replicated
    strep_d = din("strep", [Bc, NT], F32)
    enrep_d = din("enrep", [Bc, NT], F32)
    esel_d = din("esel", [128, Bc], F32)
    ohem_d = din("ohem", [128, NTILE, NT], F32)
    ohtr_d = din("ohtr", [128, NTILE, NT * NT], F32)
    ohst_d = din("ohst", [Bc, NT], F32)
    ohen_d = din("ohen", [Bc, NT], F32)
    ident_d = din("ident", [128, 128], BF16)

    scores_d = nc.dram_tensor("scores", [Bc, 1], F32, kind="ExternalOutput")
    dbg_d = nc.dram_tensor("dbg", [Bc, 2], F32, kind="ExternalOutput")

    with tile.TileContext(nc) as tc, ExitStack() as ctx:
        consts = ctx.enter_context(tc.tile_pool(name="consts", bufs=1))
        big = ctx.enter_context(tc.tile_pool(name="big", bufs=1))

        # ---- constants into SBUF
        ids_sb = consts.tile([128, NTILE], I32)
        nc.gpsimd.dma_start(out=ids_sb[:], in_=ids_d[:])
        wih_sb, whh_sb, bias_sb = {}, {}, {}
        for d in "fb":
            wih_sb[d] = consts.tile([128, 2, 4 * Hd], BF16, tag=f"wih{d}", name=f"wih{d}")
            nc.gpsimd.dma_start(out=wih_sb[d][:], in_=wih_d[d][:])
            whh_sb[d] = consts.tile([128, 2, 4 * Hd], BF16, tag=f"whh{d}", name=f"whh{d}")
            nc.gpsimd.dma_start(out=whh_sb[d][:], in_=whh_d[d][:])
            bias_sb[d] = consts.tile([128, 8], F32, tag=f"bias{d}", name=f"bias{d}")
            nc.gpsimd.dma_start(out=bias_sb[d][:], in_=bias_d[d][:])
        wout_sb = consts.tile([128, 4, TAGSET], BF16)
        nc.gpsimd.dma_start(out=wout_sb[:], in_=wout_d[:])
        boutr_sb = consts.tile([128, TAGSET], F32)
        nc.gpsimd.dma_start(out=boutr_sb[:], in_=boutr_d[:])
        trepT_sb = consts.tile([128, NT * NT], F32)
        nc.gpsimd.dma_start(out=trepT_sb[:], in_=trepT_d[:])
        trepPN_sb = consts.tile([128, NT * NT], F32)
        nc.gpsimd.dma_start(out=trepPN_sb[:], in_=trepPN_d[:])
        strep_sb = consts.tile([Bc, NT], F32)
        nc.gpsimd.dma_start(out=strep_sb[:], in_=strep_d[:])
        enrep_sb = consts.tile([Bc, NT], F32)
        nc.gpsimd.dma_start(out=enrep_sb[:], in_=enrep_d[:])
        esel_sb = consts.tile([128, Bc], F32)
        nc.gpsimd.dma_start(out=esel_sb[:], in_=esel_d[:])
        ohst_sb = consts.tile([Bc, NT], F32)
        nc.gpsimd.dma_start(out=ohst_sb[:], in_=ohst_d[:])
        ohen_sb = consts.tile([Bc, NT], F32)
        nc.gpsimd.dma_start(out=ohen_sb[:], in_=ohen_d[:])

        ident = consts.tile([128, 128], BF16)
        nc.gpsimd.dma_start(out=ident[:], in_=ident_d[:])
        hz = consts.tile([128, 2, Bc], BF16)
        nc.vector.memset(hz[:], 0.0)

        # ---- big persistent buffers
        XT = big.tile([128, 2, TOK], BF16)           # x^T (emb dim on partitions)
        ZX = {d: big.tile([128, 8, TOK], BF16, tag=f"zx{d}", name=f"zx{d}") for d in "fb"}
        H = {d: big.tile([128, 2, TOK], BF16, tag=f"h{d}", name=f"h{d}") for d in "fb"}
        em_sb = big.tile([128, NTILE, TAGSET], F32)  # emissions, token-major
        emC = big.tile([Bc, S, TAGSET], F32)         # emissions, batch-major (CRF)

        for _rep in range(repeat):
            # ---- phase B: embedding gather + cast + transpose
            with ExitStack() as pb:
                gp = pb.enter_context(tc.tile_pool(name="gp", bufs=3))
                pp = pb.enter_context(tc.tile_pool(name="pp", bufs=2, space="PSUM"))
                for i in range(NTILE):
                    xg = gp.tile([128, E], F32, tag="xg")
                    nc.gpsimd.indirect_dma_start(
                        out=xg[:],
                        out_offset=None,
                        in_=emb_d[:],
                        in_offset=bass.IndirectOffsetOnAxis(ap=ids_sb[:, i : i + 1], axis=0),
                    )
                    xc = gp.tile([128, E], BF16, tag="xc")
                    nc.vector.tensor_copy(out=xc[:], in_=xg[:])
                    for e in range(2):
                        pt = pp.tile([128, 128], BF16, tag="pt")
                        nc.tensor.transpose(
                            out=pt[:], in_=xc[:, e * 128 : (e + 1) * 128], identity=ident[:]
                        )
                        nc.vector.tensor_copy(
                            out=XT[:, e, i * 128 : (i + 1) * 128], in_=pt[:]
                        )

            # ---- phase C: input projections zx = W_ih @ x^T + bias (both dirs)
            with ExitStack() as pc:
                zp = pc.enter_context(tc.tile_pool(name="zp", bufs=2, space="PSUM"))
                for d in "fb":
                    for m in range(8):
                        for chk in range(TOK // CW):
                            zpt = zp.tile([128, CW], F32, tag="zpt")
                            for k in range(2):
                                nc.tensor.matmul(
                                    out=zpt[:],
                                    lhsT=wih_sb[d][:, k, m * 128 : (m + 1) * 128],
                                    rhs=XT[:, k, chk * CW : (chk + 1) * CW],
                                    start=(k == 0),
                                    stop=(k == 1),
                                )
                            nc.scalar.activation(
                                out=ZX[d][:, m, chk * CW : (chk + 1) * CW],
                                in_=zpt[:],
                                func=AF.Identity,
                                bias=bias_sb[d][:, m : m + 1],
                                scale=1.0,
                            )

            # ---- recurrences (fwd & bwd interleaved; weights stationary)
            with ExitStack() as pr:
                ztp = {
                    d: pr.enter_context(tc.tile_pool(name=f"zt{d}", bufs=2, space="PSUM"))
                    for d in "fb"
                }
                gw = pr.enter_context(tc.tile_pool(name="gw", bufs=3))
                gw2 = pr.enter_context(tc.tile_pool(name="gw2", bufs=3))
                cst = pr.enter_context(tc.tile_pool(name="cst", bufs=1))
                ct = {d: cst.tile([128, 2, Bc], F32, tag=f"c{d}", name=f"c{d}") for d in "fb"}
                for d in "fb":
                    nc.vector.memset(ct[d][:], 0.0)

                def lstm_step(d, t, tprev):
                    hp = hz if tprev is None else None
                    zt = ztp[d].tile([128, 8, Bc], F32, tag="zt")
                    for m in range(8):
                        for k in range(2):
                            rhs = (
                                hz[:, k, :]
                                if tprev is None
                                else H[d][:, k, tprev * Bc : (tprev + 1) * Bc]
                            )
                            nc.tensor.matmul(
                                out=zt[:, m, :],
                                lhsT=whh_sb[d][:, k, m * 128 : (m + 1) * 128],
                                rhs=rhs,
                                start=(k == 0),
                                stop=(k == 1),
                            )
                    zf = gw.tile([128, 8, Bc], F32, tag=f"zf{d}")
                    nc.vector.tensor_add(
                        out=zf[:], in0=zt[:], in1=ZX[d][:, :, t * Bc : (t + 1) * Bc]
                    )
                    # gates reordered host-side to (i, f, o, g): sigmoid on
                    # [0:6], tanh on g [6:8] -> 2 ACT ops instead of 3
                    nc.scalar.activation(out=zf[:, 0:6, :], in_=zf[:, 0:6, :], func=AF.Sigmoid)
                    nc.scalar.activation(out=zf[:, 6:8, :], in_=zf[:, 6:8, :], func=AF.Tanh)
                    a = gw2.tile([128, 2, Bc], F32, tag=f"a{d}")
                    nc.vector.tensor_mul(out=a[:], in0=zf[:, 2:4, :], in1=ct[d][:])
                    bb = gw2.tile([128, 2, Bc], F32, tag=f"b{d}")
                    nc.vector.tensor_mul(out=bb[:], in0=zf[:, 0:2, :], in1=zf[:, 6:8, :])
                    nc.vector.tensor_add(out=ct[d][:], in0=a[:], in1=bb[:])
                    tch = gw2.tile([128, 2, Bc], F32, tag=f"tc{d}")
                    nc.scalar.activation(out=tch[:], in_=ct[d][:], func=AF.Tanh)
                    nc.vector.tensor_mul(
                        out=H[d][:, :, t * Bc : (t + 1) * Bc],
                        in0=zf[:, 4:6, :],
                        in1=tch[:],
                    )

                for i in range(S):
                    lstm_step("b", S - 1 - i, None if i == 0 else S - i)
                    lstm_step("f", i, None if i == 0 else i - 1)

            # ---- emissions em = H @ w_out^T + b_out  (token-major [128, NTILE, 10])
            with ExitStack() as pe:
                ep = pe.enter_context(tc.tile_pool(name="ep", bufs=2, space="PSUM"))
                for i in range(NTILE):
                    ept = ep.tile([128, TAGSET], F32, tag="ept")
                    for k4 in range(4):
                        dsrc = "f" if k4 < 2 else "b"
                        kk = k4 % 2
                        nc.tensor.matmul(
                            out=ept[:],
                            lhsT=H[dsrc][:, kk, i * 128 : (i + 1) * 128],
                            rhs=wout_sb[:, k4, :],
                            start=(k4 == 0),
                            stop=(k4 == 3),
                        )
                    nc.vector.tensor_add(out=em_sb[:, i, :], in0=ept[:], in1=boutr_sb[:])

                # ---- numerator: gold-path score via one-hot dot products
                npool = pe.enter_context(tc.tile_pool(name="npool", bufs=3))
                nsing = pe.enter_context(tc.tile_pool(name="nsing", bufs=1))
                junk = nsing.tile([128, NT * NT], F32)
                smat = nsing.tile([128, 2 * NTILE], F32)
                for i in range(NTILE):
                    oem = npool.tile([128, NT], F32, tag="oem")
                    nc.gpsimd.dma_start(out=oem[:], in_=ohem_d[:, i, :])
                    nc.vector.tensor_mul(
                        out=junk[:, 0:NT], in0=em_sb[:, i, 1:TAGSET], in1=oem[:]
                    )
                    nc.vector.reduce_sum(
                        out=smat[:, i : i + 1], in_=junk[:, 0:NT],
                        axis=mybir.AxisListType.X,
                    )
                    otr = npool.tile([128, NT * NT], F32, tag="otr")
                    nc.gpsimd.dma_start(out=otr[:], in_=ohtr_d[:, i, :])
                    nc.vector.tensor_mul(out=junk[:], in0=otr[:], in1=trepPN_sb[:])
                    nc.vector.reduce_sum(
                        out=smat[:, NTILE + i : NTILE + i + 1], in_=junk[:],
                        axis=mybir.AxisListType.X,
                    )
                selp_pool = pe.enter_context(tc.tile_pool(name="selp", bufs=1, space="PSUM"))
                selp = selp_pool.tile([Bc, 2 * NTILE], F32)
                nc.tensor.matmul(
                    out=selp[:], lhsT=esel_sb[:], rhs=smat[:], start=True, stop=True
                )
                numm = nsing.tile([Bc, 1], F32)
                nc.vector.reduce_sum(out=numm[:], in_=selp[:], axis=mybir.AxisListType.X)
                stsc = nsing.tile([Bc, 1], F32)
                nc.vector.tensor_mul(out=junk[0:Bc, 0:NT], in0=ohst_sb[:], in1=strep_sb[:])
                nc.vector.reduce_sum(
                    out=stsc[:], in_=junk[0:Bc, 0:NT], axis=mybir.AxisListType.X
                )
                ensc = nsing.tile([Bc, 1], F32)
                nc.vector.tensor_mul(out=junk[0:Bc, 0:NT], in0=ohen_sb[:], in1=enrep_sb[:])
                nc.vector.reduce_sum(
                    out=ensc[:], in_=junk[0:Bc, 0:NT], axis=mybir.AxisListType.X
                )
                nc.vector.tensor_add(out=ensc[:], in0=ensc[:], in1=stsc[:])

                # rearrange emissions to batch-major emC[b, t, tag] so every CRF
                # operand starts at partition 0 (DVE requires same start partition)
                for r in range(TPT):
                    dst = emC[:, r, :]
                    dst3 = bass.AP(
                        tensor=dst.tensor,
                        offset=dst.offset,
                        ap=[dst.ap[0], [TPT * TAGSET, NTILE], dst.ap[1]],
                    )
                    nc.gpsimd.dma_start(
                        out=dst3, in_=em_sb[r * Bc : (r + 1) * Bc, :, :]
                    )

                # ---- CRF forward algorithm (denominator), f32, shift-stabilized
                cp = pe.enter_context(tc.tile_pool(name="cp", bufs=4))
                csing = pe.enter_context(tc.tile_pool(name="csing", bufs=1))
                Cacc = csing.tile([Bc, 1], F32)
                nc.vector.memset(Cacc[:], 0.0)

                def em_slice(t):
                    return emC[:, t, 1:TAGSET]

                alpha = cp.tile([Bc, NT], F32, tag="alpha")
                nc.vector.tensor_add(out=alpha[:], in0=strep_sb[:], in1=em_slice(0))
                for t in range(1, S):
                    nega = cp.tile([Bc, 1], F32, tag="nega")
                    nc.scalar.mul(out=nega[:], in_=alpha[:, 0:1], mul=-1.0)
                    nc.vector.tensor_add(out=Cacc[:], in0=Cacc[:], in1=alpha[:, 0:1])
                    tmp = cp.tile([Bc, NT, NT], F32, tag="tmp")
                    nc.vector.tensor_tensor(
                        out=tmp[:],
                        in0=_bcast_mid(alpha[:], NT),
                        in1=trepT_sb[0:Bc, :].rearrange("p (n m) -> p n m", n=NT),
                        op=ALU.add,
                    )
                    ex = cp.tile([Bc, NT, NT], F32, tag="ex")
                    nc.scalar.activation(
                        out=ex[:], in_=tmp[:], func=AF.Exp, bias=nega[:], scale=1.0
                    )
                    s9 = cp.tile([Bc, NT], F32, tag="s9")
                    nc.vector.reduce_sum(out=s9[:], in_=ex[:], axis=mybir.AxisListType.X)
                    l9 = cp.tile([Bc, NT], F32, tag="l9")
                    nc.scalar.activation(out=l9[:], in_=s9[:], func=AF.Ln)
                    alpha_new = cp.tile([Bc, NT], F32, tag="alpha")
                    nc.vector.tensor_add(out=alpha_new[:], in0=l9[:], in1=em_slice(t))
                    alpha = alpha_new

                # den = Cacc + logsumexp(alpha + end_trans)
                dv = csing.tile([Bc, NT], F32)
                nc.vector.tensor_add(out=dv[:], in0=alpha[:], in1=enrep_sb[:])
                mx = csing.tile([Bc, 1], F32)
                nc.vector.reduce_max(out=mx[:], in_=dv[:], axis=mybir.AxisListType.X)
                negm = csing.tile([Bc, 1], F32)
                nc.scalar.mul(out=negm[:], in_=mx[:], mul=-1.0)
                exf = csing.tile([Bc, NT], F32)
                sume = csing.tile([Bc, 1], F32)
                nc.scalar.activation(
                    out=exf[:], in_=dv[:], func=AF.Exp, bias=negm[:], scale=1.0,
                    accum_out=sume[:],
                )
                lnf = csing.tile([Bc, 1], F32)
                nc.scalar.activation(out=lnf[:], in_=sume[:], func=AF.Ln)
                den1 = csing.tile([Bc, 1], F32)
                nc.vector.tensor_add(out=den1[:], in0=mx[:], in1=lnf[:])
                den2 = csing.tile([Bc, 1], F32)
                nc.vector.tensor_add(out=den2[:], in0=den1[:], in1=Cacc[:])

                numt = csing.tile([Bc, 1], F32)
                nc.vector.tensor_add(out=numt[:], in0=numm[:], in1=ensc[:])
                sc = csing.tile([Bc, 1], F32)
                nc.vector.tensor_tensor(
                    out=sc[:], in0=numt[:], in1=den2[:], op=ALU.subtract
                )
                nc.gpsimd.dma_start(out=scores_d[:], in_=sc[:])
                dbg = csing.tile([Bc, 2], F32)
                nc.vector.tensor_copy(out=dbg[:, 0:1], in_=numt[:])
                nc.vector.tensor_copy(out=dbg[:, 1:2], in_=den2[:])
                nc.gpsimd.dma_start(out=dbg_d[:], in_=dbg[:])

    _fixup_wait_limit(nc)
    return nc


# ---------------------------------------------------------------------------
# Host side
# ---------------------------------------------------------------------------

_PROGRAM_CACHE = {}
LAST_RESULTS = None


def _get_program(S):
    if S not in _PROGRAM_CACHE:
        _PROGRAM_CACHE[S] = build_program(S)
    return _PROGRAM_CACHE[S]


def _tile_k(w, ktiles, cols):
    """[ktiles*128, cols] -> [128, ktiles, cols]"""
    return np.ascontiguousarray(
        w.reshape(ktiles, 128, cols).transpose(1, 0, 2)
    )


def _prep_common(emb_table, w_ih_f, w_hh_f, b_ih_f, b_hh_f, w_ih_b, w_hh_b,
                 b_ih_b, b_hh_b, w_out, b_out, start_trans, end_trans, trans):
    f32 = np.float32
    com = {}
    com["emb"] = np.ascontiguousarray(emb_table, dtype=f32)
    gperm = np.concatenate([
        np.arange(0, 2 * Hd),            # i, f
        np.arange(3 * Hd, 4 * Hd),       # o
        np.arange(2 * Hd, 3 * Hd),       # g
    ])
    for d, wih, whh, bi, bh in (
        ("f", w_ih_f, w_hh_f, b_ih_f, b_hh_f),
        ("b", w_ih_b, w_hh_b, b_ih_b, b_hh_b),
    ):
        com[f"wih_{d}"] = _tile_k(wih[gperm].T.astype(nbf16), 2, 4 * Hd)
        com[f"whh_{d}"] = _tile_k(whh[gperm].T.astype(nbf16), 2, 4 * Hd)
        bias = (bi + bh).astype(f32)[gperm]
        com[f"bias_{d}"] = np.ascontiguousarray(bias.reshape(8, 128).T)
    com["wout"] = _tile_k(w_out.T.astype(nbf16), 4, TAGSET)
    com["boutr"] = np.tile(b_out.astype(f32)[None, :], (128, 1))
    com["trepT"] = np.tile(trans.T.astype(f32).reshape(1, -1), (128, 1))
    com["trepPN"] = np.tile(trans.astype(f32).reshape(1, -1), (128, 1))
    com["strep"] = np.tile(start_trans.astype(f32)[None, :], (Bc, 1))
    com["enrep"] = np.tile(end_trans.astype(f32)[None, :], (Bc, 1))
    com["esel"] = (
        (np.arange(128)[:, None] % Bc) == np.arange(Bc)[None, :]
    ).astype(f32)
    com["ident"] = np.eye(128, dtype=nbf16)
    return {k: np.ascontiguousarray(v) for k, v in com.items()}


def _prep_core(inputs, tags, c, S):
    f32 = np.float32
    NTILE = S * Bc // 128
    seqs = slice(c * Bc, (c + 1) * Bc)
    # token order tau = t*Bc + b
    idmat = np.asarray(inputs[seqs]).T.astype(np.int32)       # [S, Bc]
    ids_flat = idmat.reshape(-1)                               # [S*Bc]
    ids_col = np.ascontiguousarray(ids_flat.reshape(NTILE, 128).T)  # [128, NTILE]
    tags0 = np.asarray(tags[seqs]).T.astype(np.int64) - 1      # [S, Bc], 0..8
    eye9 = np.eye(NT, dtype=f32)
    eye81 = np.eye(NT * NT, dtype=f32)
    ohem = eye9[tags0.reshape(-1)]                             # [S*Bc, 9]
    ohem = ohem.reshape(NTILE, 128, NT).transpose(1, 0, 2)
    pair = tags0[:-1] * NT + tags0[1:]                         # [S-1, Bc]
    ohtr = np.zeros((S, Bc, NT * NT), dtype=f32)
    ohtr[1:] = eye81[pair]
    ohtr = ohtr.reshape(NTILE, 128, NT * NT).transpose(1, 0, 2)
    ohst = eye9[tags0[0]]                                      # [Bc, 9]
    ohen = eye9[tags0[-1]]
    return {
        "ids": ids_col,
        "ohem": np.ascontiguousarray(ohem),
        "ohtr": np.ascontiguousarray(ohtr),
        "ohst": np.ascontiguousarray(ohst),
        "ohen": np.ascontiguousarray(ohen),
    }


def run(inputs, tags, mask, emb_table, w_ih_f, w_hh_f, b_ih_f, b_hh_f,
        w_ih_b, w_hh_b, b_ih_b, b_hh_b, w_out, b_out,
        start_trans, end_trans, trans, S=None, trace=False):
    global LAST_RESULTS
    inputs = np.asarray(inputs)
    tags = np.asarray(tags)
    if S is None:
        S = inputs.shape[1]
    nc = _get_program(S)
    com = _prep_common(
        np.asarray(emb_table), np.asarray(w_ih_f), np.asarray(w_hh_f),
        np.asarray(b_ih_f), np.asarray(b_hh_f), np.asarray(w_ih_b),
        np.asarray(w_hh_b), np.asarray(b_ih_b), np.asarray(b_hh_b),
        np.asarray(w_out), np.asarray(b_out), np.asarray(start_trans),
        np.asarray(end_trans), np.asarray(trans),
    )
    in_maps = []
    for c in range(NCORES):
        m = dict(com)
        m.update(_prep_core(inputs, tags, c, S))
        in_maps.append(m)
    res = run_bass_kernel_spmd(
        nc, in_maps, core_ids=list(range(NCORES)), trace=trace
    )
    LAST_RESULTS = res
    scores = np.concatenate([r["scores"][:, 0] for r in res.results])
    loss = -np.mean(scores)
    return np.array(loss, dtype=np.float32)


def kernel(**inputs):
    return run(**inputs)


def make_timed_runner(S, in_maps, nc=None):
    """Build a persistent jitted sharded callable with device-resident inputs
    for timing repeated executions (axon has no NTFF hook in this container)."""
    import jax
    from jax.sharding import Mesh, PartitionSpec, NamedSharding
    from jax.experimental.shard_map import shard_map
    from concourse import bass2jax as b2j
    from concourse import mybir as _mybir

    if nc is None:
        nc = _get_program(S)
    b2j.install_neuronx_cc_hook()
    partition_name = nc.partition_id_tensor.name if nc.partition_id_tensor else None
    in_names, out_names, out_avals, zero_outs = [], [], [], []
    for alloc in nc.m.functions[0].allocations:
        if not isinstance(alloc, _mybir.MemoryLocationSet):
            continue
        name = alloc.memorylocations[0].name
        if alloc.kind == "ExternalInput":
            if name != partition_name:
                in_names.append(name)
        elif alloc.kind == "ExternalOutput":
            shape = tuple(alloc.tensor_shape)
            dtype = _mybir.dt.np(alloc.dtype)
            out_names.append(name)
            out_avals.append(jax.core.ShapedArray(shape, dtype))
            zero_outs.append(np.zeros(shape, dtype))
    n_params = len(in_names)
    all_in_names = list(in_names) + list(out_names)
    if partition_name is not None:
        all_in_names.append(partition_name)

    def _body(*args):
        operands = list(args)
        if partition_name is not None:
            operands.append(b2j.partition_id_tensor())
        outs = b2j._bass_exec_p.bind(
            *operands,
            out_avals=tuple(out_avals),
            in_names=tuple(all_in_names),
            out_names=tuple(out_names),
            lowering_input_output_aliases=(),
            sim_require_finite=True,
            sim_require_nnan=True,
            nc=nc,
        )
        return tuple(outs)

    n = len(in_maps)
    devices = jax.devices()[:n]
    mesh = Mesh(np.asarray(devices), ("core",))
    in_specs = (PartitionSpec("core"),) * (n_params + len(out_names))
    out_specs = (PartitionSpec("core"),) * len(out_names)
    sharded = jax.jit(
        shard_map(_body, mesh=mesh, in_specs=in_specs, out_specs=out_specs,
                  check_rep=False),
        keep_unused=True,
    )
    sh = NamedSharding(mesh, PartitionSpec("core"))
    concat_in = [
        jax.device_put(
            np.concatenate([np.asarray(in_maps[c][nm]) for c in range(n)], axis=0), sh
        )
        for nm in in_names
    ]
    concat_zeros = [
        jax.device_put(np.zeros((n * z.shape[0], *z.shape[1:]), z.dtype), sh)
        for z in zero_outs
    ]

    def call():
        outs = sharded(*concat_in, *concat_zeros)
        jax.block_until_ready(outs)
        return outs

    return call



# revision 3
# speedup vs baseline: 51.4284x; 51.4284x over previous
"""BiLSTM-CRF loss kernel v2 for Trainium2 (8 NeuronCores, data-parallel over batch).

Differences from v1:
  - Optional on-device repeat of the whole body via tc.For_i (one instruction
    copy in the NEFF; amortizes per-call dispatch latency for timing).
  - CRF denominator in linear space: a_t = exp(em_t) (x) (a_{t-1} @ exp(trans)),
    renormalized every RENORM steps. Per step: one tiny f32 matmul + one DVE
    mul (no per-step Exp/Ln activations).
  - Emissions computed transposed ([10 tags, TOK]) via 16 wide matmuls; bias
    and exp fused into the PSUM-reading activation.
  - Input-projection add fused into the recurrence PSUM via an identity matmul
    (gates read PSUM directly from the activation engine).
"""

import numpy as np
import ml_dtypes

import concourse.bass as bass
import concourse.tile as tile
from concourse import mybir
from concourse.bass_utils import run_bass_kernel_spmd
from contextlib import ExitStack

# ---------------------------------------------------------------------------
# Workaround: this compiler build allows at most 2 sem waits on a CTRL (Drain)
# instruction; TileContext's tail drain can carry more. Split the waits across
# chained drains on the same engine.
from concourse import tile as _tile_mod
from concourse.vector_clock import ScopedClock as _ScopedClock

_MAX_DRAIN_WAITS = 1


def _split_drain_and_barrier(self, tick_clock, wait_clock):
    nc = self.nc
    drain_inst = nc.sync.drain()
    wait_clock.add_sem_waits(
        drain_inst.ins, _ScopedClock({None: tick_clock.global_clock})
    )
    si = drain_inst.ins.sync_info
    waits = list(si.on_wait or []) if si is not None else []
    if len(waits) > _MAX_DRAIN_WAITS:
        si.on_wait = waits[:_MAX_DRAIN_WAITS]
        for i in range(_MAX_DRAIN_WAITS, len(waits), _MAX_DRAIN_WAITS):
            d = nc.sync.drain()
            dsi = d.ins.sync_info
            if dsi is None:
                d.ins.sync_info = si
                dsi = d.ins.sync_info
            dsi.on_wait = waits[i : i + _MAX_DRAIN_WAITS]
            dsi.on_update = []
    nc.all_engine_barrier()
    assert self.sems is not None
    popped = nc._tile_sem_poison_stack.pop()
    assert popped is self._sem_poison
    nc.clear_and_free_semaphores(list(self.sems.allocated().values()))
    nc.all_engine_barrier()


_tile_mod.TileContext._drain_and_barrier = _split_drain_and_barrier


def _fixup_wait_limit(nc, max_waits=1):
    """Split >max_waits sem waits onto same-engine NOPs inserted before the
    offending instruction."""
    main_insts = nc.cur_bb.bb.instructions

    def make_nop(engine):
        eng = nc.engines[engine]
        bi = eng.drain(fusable=False)
        nop = bi.ins
        assert main_insts[-1].name == nop.name
        main_insts.pop()
        return nop

    from concourse import mybir as _mybir

    for f in nc.m.functions:
        for bb in f.blocks:
            insts = bb.instructions
            idx = 0
            while idx < len(insts):
                inst = insts[idx]
                si = inst.sync_info
                lim = max_waits
                waits = list(si.on_wait) if (si is not None and si.on_wait) else []
                if len(waits) > lim:
                    si.on_wait = waits[:lim]
                    excess = waits[lim:]
                    for j in range(0, len(excess), 1):
                        nop = make_nop(inst.engine)
                        nop.sync_info = _mybir.SyncInfo(
                            on_wait=excess[j : j + 1], on_update=[]
                        )
                        insts.insert(idx, nop)
                        idx += 1
                idx += 1


# ---------------------------------------------------------------------------

VOCAB = 50000
TAGSET = 10
NT = TAGSET - 1  # 9 CRF tags
E = 256
HID = 512
Hd = HID // 2  # 256 per direction
B = 64
S_FULL = 256
NCORES = 8
Bc = B // NCORES  # 8 sequences per core
RENORM = 8  # CRF linear-space renormalization period

BF16 = mybir.dt.bfloat16
F32 = mybir.dt.float32
I32 = mybir.dt.int32
AF = mybir.ActivationFunctionType
ALU = mybir.AluOpType
nbf16 = ml_dtypes.bfloat16


def _bcast_mid(ap, n):
    """AP [p, m] -> [p, n(bcast), m]"""
    return bass.AP(tensor=ap.tensor, offset=ap.offset, ap=[ap.ap[0], [0, n], ap.ap[1]])


def _view3(ap, dims):
    """Reinterpret the free dims of a 2D-sliced AP with explicit [stride, n]
    pairs. `ap` must be [p, ...]; dims is a list of [stride_elems, n]."""
    return bass.AP(tensor=ap.tensor, offset=ap.offset, ap=[ap.ap[0]] + list(dims))


def _bcast_free(ap_col, n):
    """AP [p, 1] -> [p, n] with stride-0 free dim (free-dim broadcast)."""
    return bass.AP(tensor=ap_col.tensor, offset=ap_col.offset,
                   ap=[ap_col.ap[0], [0, n]])


def build_program(S, repeat=1, phases="bcren"):
    """Build the SPMD Bass program for sequence length S (S % 16 == 0).

    phases (ablation/timing only — outputs are garbage unless all present):
    b=gather, c=inproj, r=recurrence, e=emissions+numerator, n=CRF den.
    """
    TOK = S * Bc            # tokens per core, token index tau = t*Bc + b
    NTILE = TOK // 128      # 128-token tiles
    CW = min(512, TOK)      # wide-matmul chunk width
    NCHUNK = TOK // CW

    nc = bass.Bass()

    def din(name, shape, dt):
        return nc.dram_tensor(name, shape, dt, kind="ExternalInput")

    ids_d = din("ids", [128, NTILE], I32)
    emb_d = din("emb", [VOCAB, E], F32)
    wih_d = {d: din(f"wih_{d}", [128, 2, 4 * Hd], BF16) for d in "fb"}
    whh_d = {d: din(f"whh_{d}", [128, 2, 4 * Hd], BF16) for d in "fb"}
    bias_d = {d: din(f"bias_{d}", [128, 8], F32) for d in "fb"}
    wout_d = din("wout", [128, 4, TAGSET], BF16)
    boutc_d = din("boutc", [TAGSET, 1], F32)
    etrT_d = din("etrT", [Bc, NT * NT], F32)    # exp(trans).T flat (j-major), replicated
    estrep_d = din("estrep", [Bc, NT], F32)     # exp(start_trans) replicated
    eenrep_d = din("eenrep", [Bc, NT], F32)     # exp(end_trans) replicated
    trepPN_d = din("trepPN", [128, NT * NT], F32)     # trans flat, replicated
    strep_d = din("strep", [Bc, NT], F32)
    enrep_d = din("enrep", [Bc, NT], F32)
    esel_d = din("esel", [128, Bc], F32)
    ohem_d = din("ohem", [128, NTILE * NT], F32)
    ohtr_d = din("ohtr", [128, NTILE * NT * NT], F32)
    ohst_d = din("ohst", [Bc, NT], F32)
    ohen_d = din("ohen", [Bc, NT], F32)
    ident_d = din("ident", [128, 128], BF16)

    scores_d = nc.dram_tensor("scores", [Bc, 1], F32, kind="ExternalOutput")
    dbg_d = nc.dram_tensor("dbg", [Bc, 2], F32, kind="ExternalOutput")

    with tile.TileContext(nc) as tc, ExitStack() as ctx:
        consts = ctx.enter_context(tc.tile_pool(name="consts", bufs=1))
        big = ctx.enter_context(tc.tile_pool(name="big", bufs=1))

        # ---- constants into SBUF (outside the repeat loop)
        ids_sb = consts.tile([128, NTILE], I32)
        nc.gpsimd.dma_start(out=ids_sb[:], in_=ids_d[:])
        wih_sb, whh_sb, bias_sb = {}, {}, {}
        for d in "fb":
            wih_sb[d] = consts.tile([128, 2, 4 * Hd], BF16, tag=f"wih{d}", name=f"wih{d}")
            nc.gpsimd.dma_start(out=wih_sb[d][:], in_=wih_d[d][:])
            whh_sb[d] = consts.tile([128, 2, 4 * Hd], BF16, tag=f"whh{d}", name=f"whh{d}")
            nc.gpsimd.dma_start(out=whh_sb[d][:], in_=whh_d[d][:])
            bias_sb[d] = consts.tile([128, 8], F32, tag=f"bias{d}", name=f"bias{d}")
            nc.gpsimd.dma_start(out=bias_sb[d][:], in_=bias_d[d][:])
        wout_sb = consts.tile([128, 4, TAGSET], BF16)
        nc.gpsimd.dma_start(out=wout_sb[:], in_=wout_d[:])
        boutc_sb = consts.tile([TAGSET, 1], F32)
        nc.gpsimd.dma_start(out=boutc_sb[:], in_=boutc_d[:])
        etrT_sb = consts.tile([Bc, NT * NT], F32)
        nc.gpsimd.dma_start(out=etrT_sb[:], in_=etrT_d[:])
        estrep_sb = consts.tile([Bc, NT], F32)
        nc.gpsimd.dma_start(out=estrep_sb[:], in_=estrep_d[:])
        eenrep_sb = consts.tile([Bc, NT], F32)
        nc.gpsimd.dma_start(out=eenrep_sb[:], in_=eenrep_d[:])
        trepPN_sb = consts.tile([128, NT * NT], F32)
        nc.gpsimd.dma_start(out=trepPN_sb[:], in_=trepPN_d[:])
        strep_sb = consts.tile([Bc, NT], F32)
        nc.gpsimd.dma_start(out=strep_sb[:], in_=strep_d[:])
        enrep_sb = consts.tile([Bc, NT], F32)
        nc.gpsimd.dma_start(out=enrep_sb[:], in_=enrep_d[:])
        esel_sb = consts.tile([128, Bc], F32)
        nc.gpsimd.dma_start(out=esel_sb[:], in_=esel_d[:])
        ohst_sb = consts.tile([Bc, NT], F32)
        nc.gpsimd.dma_start(out=ohst_sb[:], in_=ohst_d[:])
        ohen_sb = consts.tile([Bc, NT], F32)
        nc.gpsimd.dma_start(out=ohen_sb[:], in_=ohen_d[:])
        ohem_sb = consts.tile([128, NTILE * NT], F32)
        nc.gpsimd.dma_start(out=ohem_sb[:], in_=ohem_d[:])
        ohtr_sb = consts.tile([128, NTILE * NT * NT], F32)
        nc.gpsimd.dma_start(out=ohtr_sb[:], in_=ohtr_d[:])

        ident = consts.tile([128, 128], BF16)
        nc.gpsimd.dma_start(out=ident[:], in_=ident_d[:])
        hz = consts.tile([128, 2, Bc], BF16)
        nc.vector.memset(hz[:], 0.0)

        # ---- big persistent buffers (written each iteration)
        XT = big.tile([128, 2, TOK], BF16)           # x^T (emb dim on partitions)
        ZX = {d: big.tile([128, 8, TOK], BF16, tag=f"zx{d}", name=f"zx{d}") for d in "fb"}
        H = {d: big.tile([128, 2, TOK], BF16, tag=f"h{d}", name=f"h{d}") for d in "fb"}
        emT = big.tile([TAGSET, TOK], BF16)          # emissions^T (for transposes)
        em_sb = big.tile([128, NTILE, TAGSET], F32)  # emissions, token-major
        emC = big.tile([Bc, S, TAGSET], F32)         # emissions, batch-major
        eemC = big.tile([Bc, S, TAGSET], F32)        # exp(emissions), batch-major

        def body():
            # ---- phase B: embedding gather + cast + transpose
            if "b" not in phases:
                pass
            else:
             with ExitStack() as pb:
                gp = pb.enter_context(tc.tile_pool(name="gp", bufs=3))
                pp = pb.enter_context(tc.tile_pool(name="pp", bufs=2, space="PSUM"))
                for i in range(NTILE):
                    xg = gp.tile([128, E], F32, tag="xg")
                    nc.gpsimd.indirect_dma_start(
                        out=xg[:],
                        out_offset=None,
                        in_=emb_d[:],
                        in_offset=bass.IndirectOffsetOnAxis(ap=ids_sb[:, i : i + 1], axis=0),
                    )
                    xc = gp.tile([128, E], BF16, tag="xc")
                    nc.vector.tensor_copy(out=xc[:], in_=xg[:])
                    for e in range(2):
                        pt = pp.tile([128, 128], BF16, tag="pt")
                        nc.tensor.transpose(
                            out=pt[:], in_=xc[:, e * 128 : (e + 1) * 128], identity=ident[:]
                        )
                        nc.vector.tensor_copy(
                            out=XT[:, e, i * 128 : (i + 1) * 128], in_=pt[:]
                        )

            # ---- phase C: input projections zx = W_ih @ x^T + bias (both dirs)
            if "c" not in phases:
                pass
            else:
             with ExitStack() as pc:
                zp = pc.enter_context(tc.tile_pool(name="zp", bufs=2, space="PSUM"))
                for d in "fb":
                    for m in range(8):
                        for chk in range(NCHUNK):
                            zpt = zp.tile([128, CW], F32, tag="zpt")
                            for k in range(2):
                                nc.tensor.matmul(
                                    out=zpt[:],
                                    lhsT=wih_sb[d][:, k, m * 128 : (m + 1) * 128],
                                    rhs=XT[:, k, chk * CW : (chk + 1) * CW],
                                    start=(k == 0),
                                    stop=(k == 1),
                                )
                            nc.scalar.activation(
                                out=ZX[d][:, m, chk * CW : (chk + 1) * CW],
                                in_=zpt[:],
                                func=AF.Identity,
                                bias=bias_sb[d][:, m : m + 1],
                                scale=1.0,
                            )

            # ---- recurrences (fwd & bwd interleaved; zx added via identity MM)
            if "r" not in phases:
                pass
            else:
             with ExitStack() as pr:
                ztp = {
                    d: pr.enter_context(tc.tile_pool(name=f"zt{d}", bufs=2, space="PSUM"))
                    for d in "fb"
                }
                gw = pr.enter_context(tc.tile_pool(name="gw", bufs=3))
                gw2 = pr.enter_context(tc.tile_pool(name="gw2", bufs=3))
                cst = pr.enter_context(tc.tile_pool(name="cst", bufs=1))
                ct = {d: cst.tile([128, 2, Bc], F32, tag=f"c{d}", name=f"c{d}") for d in "fb"}
                for d in "fb":
                    nc.vector.memset(ct[d][:], 0.0)

                def lstm_step(d, t, tprev):
                    zt = ztp[d].tile([128, 8, Bc], F32, tag="zt")
                    # zx (+bias) seeds each psum accumulation group via an
                    # identity matmul; g-gates (m 6:8) first so tanh starts early
                    nc.tensor.matmul(
                        out=zt[:, 6:8, :],
                        lhsT=ident[:],
                        rhs=ZX[d][:, 6:8, t * Bc : (t + 1) * Bc],
                        start=True,
                        stop=False,
                    )
                    for m in (6, 7):
                        for k in range(2):
                            rhs = (
                                hz[:, k, :]
                                if tprev is None
                                else H[d][:, k, tprev * Bc : (tprev + 1) * Bc]
                            )
                            nc.tensor.matmul(
                                out=zt[:, m, :],
                                lhsT=whh_sb[d][:, k, m * 128 : (m + 1) * 128],
                                rhs=rhs,
                                start=False,
                                stop=(m == 7 and k == 1),
                            )
                    nc.tensor.matmul(
                        out=zt[:, 0:6, :],
                        lhsT=ident[:],
                        rhs=ZX[d][:, 0:6, t * Bc : (t + 1) * Bc],
                        start=True,
                        stop=False,
                    )
                    for m in (0, 1, 2, 3, 4, 5):
                        for k in range(2):
                            rhs = (
                                hz[:, k, :]
                                if tprev is None
                                else H[d][:, k, tprev * Bc : (tprev + 1) * Bc]
                            )
                            nc.tensor.matmul(
                                out=zt[:, m, :],
                                lhsT=whh_sb[d][:, k, m * 128 : (m + 1) * 128],
                                rhs=rhs,
                                start=False,
                                stop=(m == 5 and k == 1),
                            )
                    # gate order (i, f, o, g); ACT reads PSUM directly
                    zf = gw.tile([128, 8, Bc], F32, tag=f"zf{d}")
                    nc.scalar.activation(out=zf[:, 6:8, :], in_=zt[:, 6:8, :], func=AF.Tanh)
                    nc.scalar.activation(out=zf[:, 0:6, :], in_=zt[:, 0:6, :], func=AF.Sigmoid)
                    a = gw2.tile([128, 2, Bc], F32, tag=f"a{d}")
                    nc.vector.tensor_mul(out=a[:], in0=zf[:, 2:4, :], in1=ct[d][:])
                    bb = gw2.tile([128, 2, Bc], F32, tag=f"b{d}")
                    nc.vector.tensor_mul(out=bb[:], in0=zf[:, 0:2, :], in1=zf[:, 6:8, :])
                    nc.vector.tensor_add(out=ct[d][:], in0=a[:], in1=bb[:])
                    tch = gw2.tile([128, 2, Bc], F32, tag=f"tc{d}")
                    nc.scalar.activation(out=tch[:], in_=ct[d][:], func=AF.Tanh)
                    nc.vector.tensor_mul(
                        out=H[d][:, :, t * Bc : (t + 1) * Bc],
                        in0=zf[:, 4:6, :],
                        in1=tch[:],
                    )

                for i in range(S):
                    lstm_step("b", S - 1 - i, None if i == 0 else S - i)
                    lstm_step("f", i, None if i == 0 else i - 1)

            # ---- emissions + numerator + CRF denominator
            if "e" not in phases:
                pass
            else:
             with ExitStack() as pen:
                nsing = pen.enter_context(tc.tile_pool(name="nsing", bufs=1))
                csing = pen.enter_context(tc.tile_pool(name="csing", bufs=1))

                with ExitStack() as pe1:
                    # emissions: emT [10, TOK] = w_out @ h^T + b_out (wide chunks)
                    ep = pe1.enter_context(tc.tile_pool(name="ep", bufs=2, space="PSUM"))
                    et = pe1.enter_context(tc.tile_pool(name="et", bufs=2, space="PSUM"))
                    for chk in range(NCHUNK):
                        ept = ep.tile([TAGSET, CW], F32, tag="ept")
                        for k4 in range(4):
                            dsrc = "f" if k4 < 2 else "b"
                            kk = k4 % 2
                            nc.tensor.matmul(
                                out=ept[:],
                                lhsT=wout_sb[:, k4, :],
                                rhs=H[dsrc][:, kk, chk * CW : (chk + 1) * CW],
                                start=(k4 == 0),
                                stop=(k4 == 3),
                            )
                        # raw emissions (bf16, for the numerator via transposes)
                        nc.scalar.activation(
                            out=emT[:, chk * CW : (chk + 1) * CW], in_=ept[:],
                            func=AF.Identity, bias=boutc_sb[:], scale=1.0,
                        )

                    # token-major emissions (numerator + batch-major rearrange)
                    for i in range(NTILE):
                        pt2 = et.tile([128, TAGSET], BF16, tag="pt2")
                        nc.tensor.transpose(
                            out=pt2[:], in_=emT[:, i * 128 : (i + 1) * 128],
                            identity=ident[0:TAGSET, 0:TAGSET],
                        )
                        nc.vector.tensor_copy(out=em_sb[:, i, :], in_=pt2[:])

                    # batch-major rearrange: partition p holds token t*Bc+b with
                    # b = p%Bc, r = p//Bc; tile i covers t = i*TPT + r
                    TPT = 128 // Bc
                    for r in range(TPT):
                        dst = emC[:, r, :]
                        dst3 = bass.AP(
                            tensor=dst.tensor,
                            offset=dst.offset,
                            ap=[dst.ap[0], [TPT * TAGSET, NTILE], dst.ap[1]],
                        )
                        nc.gpsimd.dma_start(
                            out=dst3, in_=em_sb[r * Bc : (r + 1) * Bc, :, :]
                        )
                    # exp over the whole batch-major emissions in one pass
                    nc.scalar.activation(out=eemC[:], in_=emC[:], func=AF.Exp)

                    # numerator: gold-path score via batched one-hot dot products
                    npool = pe1.enter_context(tc.tile_pool(name="npool", bufs=1))
                    junk = npool.tile([128, NTILE * NT * NT], F32)
                    junk2 = npool.tile([128, NTILE * NT], F32)
                    smat = npool.tile([128, 2 * NTILE], F32)
                    nc.vector.tensor_tensor(
                        out=_view3(junk2[:], [[NT, NTILE], [1, NT]]),
                        in0=em_sb[:, :, 1:TAGSET],
                        in1=_view3(ohem_sb[:], [[NT, NTILE], [1, NT]]),
                        op=ALU.mult,
                    )
                    nc.vector.reduce_sum(
                        out=smat[:, 0:NTILE],
                        in_=_view3(junk2[:], [[NT, NTILE], [1, NT]]),
                        axis=mybir.AxisListType.X,
                    )
                    nc.vector.tensor_tensor(
                        out=_view3(junk[:], [[NT * NT, NTILE], [1, NT * NT]]),
                        in0=_view3(ohtr_sb[:], [[NT * NT, NTILE], [1, NT * NT]]),
                        in1=_bcast_mid(trepPN_sb[:], NTILE),
                        op=ALU.mult,
                    )
                    nc.vector.reduce_sum(
                        out=smat[:, NTILE : 2 * NTILE],
                        in_=_view3(junk[:], [[NT * NT, NTILE], [1, NT * NT]]),
                        axis=mybir.AxisListType.X,
                    )
                    selp_pool = pe1.enter_context(
                        tc.tile_pool(name="selp", bufs=1, space="PSUM"))
                    selp = selp_pool.tile([Bc, 2 * NTILE], F32)
                    nc.tensor.matmul(
                        out=selp[:], lhsT=esel_sb[:], rhs=smat[:], start=True, stop=True
                    )
                    numm = nsing.tile([Bc, 1], F32)
                    nc.vector.reduce_sum(out=numm[:], in_=selp[:], axis=mybir.AxisListType.X)
                    stsc = nsing.tile([Bc, 1], F32)
                    jsmall = nsing.tile([Bc, NT], F32)
                    nc.vector.tensor_mul(out=jsmall[:], in0=ohst_sb[:], in1=strep_sb[:])
                    nc.vector.reduce_sum(out=stsc[:], in_=jsmall[:], axis=mybir.AxisListType.X)
                    ensc = nsing.tile([Bc, 1], F32)
                    nc.vector.tensor_mul(out=jsmall[:], in0=ohen_sb[:], in1=enrep_sb[:])
                    nc.vector.reduce_sum(out=ensc[:], in_=jsmall[:], axis=mybir.AxisListType.X)
                    nc.vector.tensor_add(out=ensc[:], in0=ensc[:], in1=stsc[:])
                    numt = nsing.tile([Bc, 1], F32)
                    nc.vector.tensor_add(out=numt[:], in0=numm[:], in1=ensc[:])

                # CRF denominator: linear-space forward, batch-major, all-DVE
                # a_t[b, j] = eem[b, t, j] * sum_i a_{t-1}[b, i] * exp(tr)[i, j]
                if "n" not in phases:
                    pass
                else:
                 with ExitStack() as pe2:
                    cp = pe2.enter_context(tc.tile_pool(name="cp", bufs=4))
                    cacc = csing.tile([Bc, 1], F32)
                    nc.vector.memset(cacc[:], 0.0)
                    etrT3 = _view3(etrT_sb[:], [[NT, NT], [1, NT]])  # [Bc, j, i]

                    a = cp.tile([Bc, NT], F32, tag="a")
                    nc.vector.tensor_mul(
                        out=a[:], in0=estrep_sb[:], in1=eemC[:, 0, 1:TAGSET]
                    )
                    for t in range(1, S):
                        tmp = cp.tile([Bc, NT, NT], F32, tag="tmp")
                        nc.vector.tensor_tensor(
                            out=tmp[:], in0=_bcast_mid(a[:], NT), in1=etrT3,
                            op=ALU.mult,
                        )
                        a_new = cp.tile([Bc, NT], F32, tag="a")
                        nc.vector.reduce_sum(
                            out=a_new[:], in_=tmp[:], axis=mybir.AxisListType.X
                        )
                        a_em = cp.tile([Bc, NT], F32, tag="a")
                        nc.vector.tensor_mul(
                            out=a_em[:], in0=a_new[:], in1=eemC[:, t, 1:TAGSET]
                        )
                        a = a_em
                        if t % RENORM == RENORM - 1 or t == S - 1:
                            cs = cp.tile([Bc, 1], F32, tag="cs")
                            nc.vector.reduce_sum(
                                out=cs[:], in_=a[:], axis=mybir.AxisListType.X
                            )
                            # ln(cs) + accumulate runs off the critical path (ACT)
                            lncs = cp.tile([Bc, 1], F32, tag="lncs")
                            nc.scalar.activation(out=lncs[:], in_=cs[:], func=AF.Ln)
                            nc.vector.tensor_add(out=cacc[:], in0=cacc[:], in1=lncs[:])
                            rec = cp.tile([Bc, 1], F32, tag="rec")
                            nc.vector.reciprocal(out=rec[:], in_=cs[:])
                            a_rn = cp.tile([Bc, NT], F32, tag="a")
                            nc.vector.tensor_tensor(
                                out=a_rn[:], in0=a[:], in1=_bcast_free(rec[:], NT),
                                op=ALU.mult,
                            )
                            a = a_rn

                    # den = cacc + ln(sum_j a[j] * eend[j])
                    af = csing.tile([Bc, NT], F32)
                    nc.vector.tensor_mul(out=af[:], in0=a[:], in1=eenrep_sb[:])
                    fs = csing.tile([Bc, 1], F32)
                    nc.vector.reduce_sum(out=fs[:], in_=af[:], axis=mybir.AxisListType.X)
                    lnf = csing.tile([Bc, 1], F32)
                    nc.scalar.activation(out=lnf[:], in_=fs[:], func=AF.Ln)
                    den = csing.tile([Bc, 1], F32)
                    nc.vector.tensor_add(out=den[:], in0=lnf[:], in1=cacc[:])

                    sc = csing.tile([Bc, 1], F32)
                    nc.vector.tensor_tensor(
                        out=sc[:], in0=numt[:], in1=den[:], op=ALU.subtract
                    )
                    nc.gpsimd.dma_start(out=scores_d[:], in_=sc[:])
                    dbg = csing.tile([Bc, 2], F32)
                    nc.vector.tensor_copy(out=dbg[:, 0:1], in_=numt[:])
                    nc.vector.tensor_copy(out=dbg[:, 1:2], in_=den[:])
                    nc.gpsimd.dma_start(out=dbg_d[:], in_=dbg[:])

        # hardware loops (tc.For_i) fail this walrus build's codegen
        # ("ISA wrong length"), so repeat is python-unrolled
        for _rep in range(repeat):
            body()

    _fixup_wait_limit(nc)
    return nc


# ---------------------------------------------------------------------------
# Host side
# ---------------------------------------------------------------------------

_PROGRAM_CACHE = {}
LAST_RESULTS = None


def _get_program(S, repeat=1):
    key = (S, repeat)
    if key not in _PROGRAM_CACHE:
        _PROGRAM_CACHE[key] = build_program(S, repeat=repeat)
    return _PROGRAM_CACHE[key]


def _tile_k(w, ktiles, cols):
    """[ktiles*128, cols] -> [128, ktiles, cols]"""
    return np.ascontiguousarray(w.reshape(ktiles, 128, cols).transpose(1, 0, 2))


def _prep_common(emb_table, w_ih_f, w_hh_f, b_ih_f, b_hh_f, w_ih_b, w_hh_b,
                 b_ih_b, b_hh_b, w_out, b_out, start_trans, end_trans, trans):
    f32 = np.float32
    com = {}
    com["emb"] = np.ascontiguousarray(emb_table, dtype=f32)
    gperm = np.concatenate([
        np.arange(0, 2 * Hd),            # i, f
        np.arange(3 * Hd, 4 * Hd),       # o
        np.arange(2 * Hd, 3 * Hd),       # g
    ])
    for d, wih, whh, bi, bh in (
        ("f", w_ih_f, w_hh_f, b_ih_f, b_hh_f),
        ("b", w_ih_b, w_hh_b, b_ih_b, b_hh_b),
    ):
        com[f"wih_{d}"] = _tile_k(wih[gperm].T.astype(nbf16), 2, 4 * Hd)
        com[f"whh_{d}"] = _tile_k(whh[gperm].T.astype(nbf16), 2, 4 * Hd)
        bias = (bi + bh).astype(f32)[gperm]
        com[f"bias_{d}"] = np.ascontiguousarray(bias.reshape(8, 128).T)
    com["wout"] = _tile_k(w_out.T.astype(nbf16), 4, TAGSET)
    com["boutc"] = np.ascontiguousarray(b_out.astype(f32)[:, None])
    com["etrT"] = np.tile(
        np.exp(trans.astype(f32)).T.reshape(1, -1), (Bc, 1))  # [Bc, j*9+i]
    com["estrep"] = np.tile(np.exp(start_trans.astype(f32))[None, :], (Bc, 1))
    com["eenrep"] = np.tile(np.exp(end_trans.astype(f32))[None, :], (Bc, 1))
    com["trepPN"] = np.tile(trans.astype(f32).reshape(1, -1), (128, 1))
    com["strep"] = np.tile(start_trans.astype(f32)[None, :], (Bc, 1))
    com["enrep"] = np.tile(end_trans.astype(f32)[None, :], (Bc, 1))
    com["esel"] = (
        (np.arange(128)[:, None] % Bc) == np.arange(Bc)[None, :]
    ).astype(f32)
    com["ident"] = np.eye(128, dtype=nbf16)
    return {k: np.ascontiguousarray(v) for k, v in com.items()}


def _prep_core(inputs, tags, c, S):
    f32 = np.float32
    NTILE = S * Bc // 128
    seqs = slice(c * Bc, (c + 1) * Bc)
    # token order tau = t*Bc + b
    idmat = np.asarray(inputs[seqs]).T.astype(np.int32)       # [S, Bc]
    ids_flat = idmat.reshape(-1)                               # [S*Bc]
    ids_col = np.ascontiguousarray(ids_flat.reshape(NTILE, 128).T)  # [128, NTILE]
    tags0 = np.asarray(tags[seqs]).T.astype(np.int64) - 1      # [S, Bc], 0..8
    eye9 = np.eye(NT, dtype=f32)
    eye81 = np.eye(NT * NT, dtype=f32)
    ohem = eye9[tags0.reshape(-1)]                             # [S*Bc, 9]
    ohem = ohem.reshape(NTILE, 128, NT).transpose(1, 0, 2)
    pair = tags0[:-1] * NT + tags0[1:]                         # [S-1, Bc]
    ohtr = np.zeros((S, Bc, NT * NT), dtype=f32)
    ohtr[1:] = eye81[pair]
    ohtr = ohtr.reshape(NTILE, 128, NT * NT).transpose(1, 0, 2)
    ohst = eye9[tags0[0]]                                      # [Bc, 9]
    ohen = eye9[tags0[-1]]
    return {
        "ids": ids_col,
        "ohem": np.ascontiguousarray(ohem.reshape(128, -1)),
        "ohtr": np.ascontiguousarray(ohtr.reshape(128, -1)),
        "ohst": ohst,
        "ohen": ohen,
    }


def _make_in_maps(inputs, tags, mask, emb_table, w_ih_f, w_hh_f, b_ih_f, b_hh_f,
                  w_ih_b, w_hh_b, b_ih_b, b_hh_b, w_out, b_out,
                  start_trans, end_trans, trans, S):
    com = _prep_common(
        np.asarray(emb_table), np.asarray(w_ih_f), np.asarray(w_hh_f),
        np.asarray(b_ih_f), np.asarray(b_hh_f), np.asarray(w_ih_b),
        np.asarray(w_hh_b), np.asarray(b_ih_b), np.asarray(b_hh_b),
        np.asarray(w_out), np.asarray(b_out), np.asarray(start_trans),
        np.asarray(end_trans), np.asarray(trans),
    )
    in_maps = []
    for c in range(NCORES):
        m = dict(com)
        m.update(_prep_core(np.asarray(inputs), np.asarray(tags), c, S))
        in_maps.append(m)
    return in_maps


def run(inputs, tags, mask, emb_table, w_ih_f, w_hh_f, b_ih_f, b_hh_f,
        w_ih_b, w_hh_b, b_ih_b, b_hh_b, w_out, b_out,
        start_trans, end_trans, trans, S=None, trace=False):
    global LAST_RESULTS
    inputs = np.asarray(inputs)
    if S is None:
        S = inputs.shape[1]
    nc = _get_program(S)
    in_maps = _make_in_maps(
        inputs, tags, mask, emb_table, w_ih_f, w_hh_f, b_ih_f, b_hh_f,
        w_ih_b, w_hh_b, b_ih_b, b_hh_b, w_out, b_out,
        start_trans, end_trans, trans, S)
    res = run_bass_kernel_spmd(
        nc, in_maps, core_ids=list(range(NCORES)), trace=trace
    )
    LAST_RESULTS = res
    scores = np.concatenate([r["scores"][:, 0] for r in res.results])
    loss = -np.mean(scores)
    return np.array(loss, dtype=np.float32)


def kernel(**inputs):
    return run(**inputs)


def make_timed_runner(S, in_maps, nc=None, repeat=1):
    """Build a persistent jitted sharded callable with device-resident inputs
    for timing repeated executions (axon has no NTFF hook in this container)."""
    import jax
    from jax.sharding import Mesh, PartitionSpec, NamedSharding
    from jax.experimental.shard_map import shard_map
    from concourse import bass2jax as b2j
    from concourse import mybir as _mybir

    if nc is None:
        nc = _get_program(S, repeat=repeat)
    b2j.install_neuronx_cc_hook()
    partition_name = nc.partition_id_tensor.name if nc.partition_id_tensor else None
    in_names, out_names, out_avals, zero_outs = [], [], [], []
    for alloc in nc.m.functions[0].allocations:
        if not isinstance(alloc, _mybir.MemoryLocationSet):
            continue
        name = alloc.memorylocations[0].name
        if alloc.kind == "ExternalInput":
            if name != partition_name:
                in_names.append(name)
        elif alloc.kind == "ExternalOutput":
            shape = tuple(alloc.tensor_shape)
            dtype = _mybir.dt.np(alloc.dtype)
            out_names.append(name)
            out_avals.append(jax.core.ShapedArray(shape, dtype))
            zero_outs.append(np.zeros(shape, dtype))
    n_params = len(in_names)
    all_in_names = list(in_names) + list(out_names)
    if partition_name is not None:
        all_in_names.append(partition_name)

    def _body(*args):
        operands = list(args)
        if partition_name is not None:
            operands.append(b2j.partition_id_tensor())
        outs = b2j._bass_exec_p.bind(
            *operands,
            out_avals=tuple(out_avals),
            in_names=tuple(all_in_names),
            out_names=tuple(out_names),
            lowering_input_output_aliases=(),
            sim_require_finite=True,
            sim_require_nnan=True,
            nc=nc,
        )
        return tuple(outs)

    n = len(in_maps)
    devices = jax.devices()[:n]
    mesh = Mesh(np.asarray(devices), ("core",))
    in_specs = (PartitionSpec("core"),) * (n_params + len(out_names))
    out_specs = (PartitionSpec("core"),) * len(out_names)
    sharded = jax.jit(
        shard_map(_body, mesh=mesh, in_specs=in_specs, out_specs=out_specs,
                  check_rep=False),
        keep_unused=True,
    )
    sh = NamedSharding(mesh, PartitionSpec("core"))
    concat_in = [
        jax.device_put(
            np.concatenate([np.asarray(in_maps[c][nm]) for c in range(n)], axis=0), sh
        )
        for nm in in_names
    ]
    concat_zeros = [
        jax.device_put(np.zeros((n * z.shape[0], *z.shape[1:]), z.dtype), sh)
        for z in zero_outs
    ]

    def call():
        outs = sharded(*concat_in, *concat_zeros)
        jax.block_until_ready(outs)
        return outs

    def call_async(n):
        """Dispatch n executions back-to-back; block only on the last.
        The axon client pipelines the execute RPCs, so the per-call
        tunnel latency amortizes across all n."""
        outs = None
        for _ in range(n):
            outs = sharded(*concat_in, *concat_zeros)
        jax.block_until_ready(outs)
        return outs

    call.out_names = out_names
    call.call_async = call_async
    return call


# revision 4
# speedup vs baseline: 51.6070x; 1.0035x over previous
"""BiLSTM-CRF loss kernel v2 for Trainium2 (8 NeuronCores, data-parallel over batch).

Differences from v1:
  - Optional on-device repeat of the whole body via tc.For_i (one instruction
    copy in the NEFF; amortizes per-call dispatch latency for timing).
  - CRF denominator in linear space: a_t = exp(em_t) (x) (a_{t-1} @ exp(trans)),
    renormalized every RENORM steps. Per step: one tiny f32 matmul + one DVE
    mul (no per-step Exp/Ln activations).
  - Emissions computed transposed ([10 tags, TOK]) via 16 wide matmuls; bias
    and exp fused into the PSUM-reading activation.
  - Input-projection add fused into the recurrence PSUM via an identity matmul
    (gates read PSUM directly from the activation engine).
"""

import numpy as np
import ml_dtypes

import concourse.bass as bass
import concourse.tile as tile
from concourse import mybir
from concourse.bass_utils import run_bass_kernel_spmd
from contextlib import ExitStack

# ---------------------------------------------------------------------------
# Workaround: this compiler build allows at most 2 sem waits on a CTRL (Drain)
# instruction; TileContext's tail drain can carry more. Split the waits across
# chained drains on the same engine.
from concourse import tile as _tile_mod
from concourse.vector_clock import ScopedClock as _ScopedClock

_MAX_DRAIN_WAITS = 1


def _split_drain_and_barrier(self, tick_clock, wait_clock):
    nc = self.nc
    drain_inst = nc.sync.drain()
    wait_clock.add_sem_waits(
        drain_inst.ins, _ScopedClock({None: tick_clock.global_clock})
    )
    si = drain_inst.ins.sync_info
    waits = list(si.on_wait or []) if si is not None else []
    if len(waits) > _MAX_DRAIN_WAITS:
        si.on_wait = waits[:_MAX_DRAIN_WAITS]
        for i in range(_MAX_DRAIN_WAITS, len(waits), _MAX_DRAIN_WAITS):
            d = nc.sync.drain()
            dsi = d.ins.sync_info
            if dsi is None:
                d.ins.sync_info = si
                dsi = d.ins.sync_info
            dsi.on_wait = waits[i : i + _MAX_DRAIN_WAITS]
            dsi.on_update = []
    nc.all_engine_barrier()
    assert self.sems is not None
    popped = nc._tile_sem_poison_stack.pop()
    assert popped is self._sem_poison
    nc.clear_and_free_semaphores(list(self.sems.allocated().values()))
    nc.all_engine_barrier()


_tile_mod.TileContext._drain_and_barrier = _split_drain_and_barrier


def _fixup_wait_limit(nc, max_waits=1):
    """Split >max_waits sem waits onto same-engine NOPs inserted before the
    offending instruction."""
    main_insts = nc.cur_bb.bb.instructions

    def make_nop(engine):
        eng = nc.engines[engine]
        bi = eng.drain(fusable=False)
        nop = bi.ins
        assert main_insts[-1].name == nop.name
        main_insts.pop()
        return nop

    from concourse import mybir as _mybir

    for f in nc.m.functions:
        for bb in f.blocks:
            insts = bb.instructions
            idx = 0
            while idx < len(insts):
                inst = insts[idx]
                si = inst.sync_info
                lim = max_waits
                waits = list(si.on_wait) if (si is not None and si.on_wait) else []
                if len(waits) > lim:
                    si.on_wait = waits[:lim]
                    excess = waits[lim:]
                    for j in range(0, len(excess), 1):
                        nop = make_nop(inst.engine)
                        nop.sync_info = _mybir.SyncInfo(
                            on_wait=excess[j : j + 1], on_update=[]
                        )
                        insts.insert(idx, nop)
                        idx += 1
                idx += 1


# ---------------------------------------------------------------------------

VOCAB = 50000
TAGSET = 10
NT = TAGSET - 1  # 9 CRF tags
E = 256
HID = 512
Hd = HID // 2  # 256 per direction
B = 64
S_FULL = 256
NCORES = 8
Bc = B // NCORES  # 8 sequences per core
RENORM = 8  # CRF linear-space renormalization period

BF16 = mybir.dt.bfloat16
F32 = mybir.dt.float32
I32 = mybir.dt.int32
AF = mybir.ActivationFunctionType
ALU = mybir.AluOpType
nbf16 = ml_dtypes.bfloat16


def _bcast_mid(ap, n):
    """AP [p, m] -> [p, n(bcast), m]"""
    return bass.AP(tensor=ap.tensor, offset=ap.offset, ap=[ap.ap[0], [0, n], ap.ap[1]])


def _view3(ap, dims):
    """Reinterpret the free dims of a 2D-sliced AP with explicit [stride, n]
    pairs. `ap` must be [p, ...]; dims is a list of [stride_elems, n]."""
    return bass.AP(tensor=ap.tensor, offset=ap.offset, ap=[ap.ap[0]] + list(dims))


def _bcast_free(ap_col, n):
    """AP [p, 1] -> [p, n] with stride-0 free dim (free-dim broadcast)."""
    return bass.AP(tensor=ap_col.tensor, offset=ap_col.offset,
                   ap=[ap_col.ap[0], [0, n]])


def build_program(S, repeat=1, phases="bcren"):
    """Build the SPMD Bass program for sequence length S (S % 16 == 0).

    phases (ablation/timing only — outputs are garbage unless all present):
    b=gather, c=inproj, r=recurrence, e=emissions+numerator, n=CRF den.
    """
    TOK = S * Bc            # tokens per core, token index tau = t*Bc + b
    NTILE = TOK // 128      # 128-token tiles
    CW = min(512, TOK)      # wide-matmul chunk width
    NCHUNK = TOK // CW

    nc = bass.Bass()

    def din(name, shape, dt):
        return nc.dram_tensor(name, shape, dt, kind="ExternalInput")

    ids_d = din("ids", [128, NTILE], I32)
    emb_d = din("emb", [VOCAB, E], F32)
    wih_d = {d: din(f"wih_{d}", [128, 2, 4 * Hd], BF16) for d in "fb"}
    whh_d = {d: din(f"whh_{d}", [128, 2, 4 * Hd], BF16) for d in "fb"}
    bias_d = {d: din(f"bias_{d}", [128, 8], F32) for d in "fb"}
    wout_d = din("wout", [128, 4, TAGSET], BF16)
    boutc_d = din("boutc", [TAGSET, 1], F32)
    etrT_d = din("etrT", [Bc, NT * NT], F32)    # exp(trans).T flat (j-major), replicated
    estrep_d = din("estrep", [Bc, NT], F32)     # exp(start_trans) replicated
    eenrep_d = din("eenrep", [Bc, NT], F32)     # exp(end_trans) replicated
    trepPN_d = din("trepPN", [128, NT * NT], F32)     # trans flat, replicated
    strep_d = din("strep", [Bc, NT], F32)
    enrep_d = din("enrep", [Bc, NT], F32)
    esel_d = din("esel", [128, Bc], F32)
    ohem_d = din("ohem", [128, NTILE * NT], F32)
    ohtr_d = din("ohtr", [128, NTILE * NT * NT], F32)
    ohst_d = din("ohst", [Bc, NT], F32)
    ohen_d = din("ohen", [Bc, NT], F32)
    ident_d = din("ident", [128, 128], BF16)

    scores_d = nc.dram_tensor("scores", [Bc, 1], F32, kind="ExternalOutput")
    dbg_d = nc.dram_tensor("dbg", [Bc, 2], F32, kind="ExternalOutput")

    with tile.TileContext(nc) as tc, ExitStack() as ctx:
        consts = ctx.enter_context(tc.tile_pool(name="consts", bufs=1))
        big = ctx.enter_context(tc.tile_pool(name="big", bufs=1))

        # ---- constants into SBUF (outside the repeat loop)
        ids_sb = consts.tile([128, NTILE], I32)
        nc.gpsimd.dma_start(out=ids_sb[:], in_=ids_d[:])
        wih_sb, whh_sb, bias_sb = {}, {}, {}
        for d in "fb":
            wih_sb[d] = consts.tile([128, 2, 4 * Hd], BF16, tag=f"wih{d}", name=f"wih{d}")
            nc.gpsimd.dma_start(out=wih_sb[d][:], in_=wih_d[d][:])
            whh_sb[d] = consts.tile([128, 2, 4 * Hd], BF16, tag=f"whh{d}", name=f"whh{d}")
            nc.gpsimd.dma_start(out=whh_sb[d][:], in_=whh_d[d][:])
            bias_sb[d] = consts.tile([128, 8], F32, tag=f"bias{d}", name=f"bias{d}")
            nc.gpsimd.dma_start(out=bias_sb[d][:], in_=bias_d[d][:])
        wout_sb = consts.tile([128, 4, TAGSET], BF16)
        nc.gpsimd.dma_start(out=wout_sb[:], in_=wout_d[:])
        boutc_sb = consts.tile([TAGSET, 1], F32)
        nc.gpsimd.dma_start(out=boutc_sb[:], in_=boutc_d[:])
        etrT_sb = consts.tile([Bc, NT * NT], F32)
        nc.gpsimd.dma_start(out=etrT_sb[:], in_=etrT_d[:])
        estrep_sb = consts.tile([Bc, NT], F32)
        nc.gpsimd.dma_start(out=estrep_sb[:], in_=estrep_d[:])
        eenrep_sb = consts.tile([Bc, NT], F32)
        nc.gpsimd.dma_start(out=eenrep_sb[:], in_=eenrep_d[:])
        trepPN_sb = consts.tile([128, NT * NT], F32)
        nc.gpsimd.dma_start(out=trepPN_sb[:], in_=trepPN_d[:])
        strep_sb = consts.tile([Bc, NT], F32)
        nc.gpsimd.dma_start(out=strep_sb[:], in_=strep_d[:])
        enrep_sb = consts.tile([Bc, NT], F32)
        nc.gpsimd.dma_start(out=enrep_sb[:], in_=enrep_d[:])
        esel_sb = consts.tile([128, Bc], F32)
        nc.gpsimd.dma_start(out=esel_sb[:], in_=esel_d[:])
        ohst_sb = consts.tile([Bc, NT], F32)
        nc.gpsimd.dma_start(out=ohst_sb[:], in_=ohst_d[:])
        ohen_sb = consts.tile([Bc, NT], F32)
        nc.gpsimd.dma_start(out=ohen_sb[:], in_=ohen_d[:])
        ohem_sb = consts.tile([128, NTILE * NT], F32)
        nc.gpsimd.dma_start(out=ohem_sb[:], in_=ohem_d[:])
        ohtr_sb = consts.tile([128, NTILE * NT * NT], F32)
        nc.gpsimd.dma_start(out=ohtr_sb[:], in_=ohtr_d[:])

        ident = consts.tile([128, 128], BF16)
        nc.gpsimd.dma_start(out=ident[:], in_=ident_d[:])
        hz = consts.tile([128, 2, Bc], BF16)
        nc.vector.memset(hz[:], 0.0)

        # ---- big per-iteration buffers. XT/ZX single-buffered; the H /
        # emissions chain double-buffered so iteration k's emissions + CRF
        # overlap iteration k+1's recurrence in the unrolled repeat stream.
        big2 = ctx.enter_context(tc.tile_pool(name="big2", bufs=2))

        def body():
            XT = big.tile([128, 2, TOK], BF16, tag="XT", name="XT")
            ZX = {d: big.tile([128, 8, TOK], BF16, tag=f"zx{d}", name=f"zx{d}")
                  for d in "fb"}
            H = {d: big2.tile([128, 2, TOK], BF16, tag=f"h{d}", name=f"h{d}")
                 for d in "fb"}
            emT = big2.tile([TAGSET, TOK], BF16, tag="emT", name="emT")
            em_sb = big2.tile([128, NTILE, TAGSET], F32, tag="em_sb", name="em_sb")
            emC = big2.tile([Bc, S, TAGSET], F32, tag="emC", name="emC")
            eemC = big2.tile([Bc, S, TAGSET], F32, tag="eemC", name="eemC")
            # ---- phase B: embedding gather + cast + transpose
            if "b" not in phases:
                pass
            else:
             with ExitStack() as pb:
                gp = pb.enter_context(tc.tile_pool(name="gp", bufs=3))
                pp = pb.enter_context(tc.tile_pool(name="pp", bufs=2, space="PSUM"))
                for i in range(NTILE):
                    xg = gp.tile([128, E], F32, tag="xg")
                    nc.gpsimd.indirect_dma_start(
                        out=xg[:],
                        out_offset=None,
                        in_=emb_d[:],
                        in_offset=bass.IndirectOffsetOnAxis(ap=ids_sb[:, i : i + 1], axis=0),
                    )
                    xc = gp.tile([128, E], BF16, tag="xc")
                    nc.vector.tensor_copy(out=xc[:], in_=xg[:])
                    for e in range(2):
                        pt = pp.tile([128, 128], BF16, tag="pt")
                        nc.tensor.transpose(
                            out=pt[:], in_=xc[:, e * 128 : (e + 1) * 128], identity=ident[:]
                        )
                        nc.vector.tensor_copy(
                            out=XT[:, e, i * 128 : (i + 1) * 128], in_=pt[:]
                        )

            # ---- phase C: input projections zx = W_ih @ x^T + bias (both dirs)
            if "c" not in phases:
                pass
            else:
             with ExitStack() as pc:
                zp = pc.enter_context(tc.tile_pool(name="zp", bufs=2, space="PSUM"))
                for d in "fb":
                    for m in range(8):
                        for chk in range(NCHUNK):
                            zpt = zp.tile([128, CW], F32, tag="zpt")
                            for k in range(2):
                                nc.tensor.matmul(
                                    out=zpt[:],
                                    lhsT=wih_sb[d][:, k, m * 128 : (m + 1) * 128],
                                    rhs=XT[:, k, chk * CW : (chk + 1) * CW],
                                    start=(k == 0),
                                    stop=(k == 1),
                                )
                            nc.scalar.activation(
                                out=ZX[d][:, m, chk * CW : (chk + 1) * CW],
                                in_=zpt[:],
                                func=AF.Identity,
                                bias=bias_sb[d][:, m : m + 1],
                                scale=1.0,
                            )

            # ---- recurrences (fwd & bwd interleaved; zx added via identity MM)
            if "r" not in phases:
                pass
            else:
             with ExitStack() as pr:
                ztp = {
                    d: pr.enter_context(tc.tile_pool(name=f"zt{d}", bufs=2, space="PSUM"))
                    for d in "fb"
                }
                gw = pr.enter_context(tc.tile_pool(name="gw", bufs=3))
                gw2 = pr.enter_context(tc.tile_pool(name="gw2", bufs=3))
                cst = pr.enter_context(tc.tile_pool(name="cst", bufs=1))
                ct = {d: cst.tile([128, 2, Bc], F32, tag=f"c{d}", name=f"c{d}") for d in "fb"}
                for d in "fb":
                    nc.vector.memset(ct[d][:], 0.0)

                def lstm_step(d, t, tprev):
                    zt = ztp[d].tile([128, 8, Bc], F32, tag="zt")
                    # zx (+bias) seeds each psum accumulation group via an
                    # identity matmul; g-gates (m 6:8) first so tanh starts early
                    nc.tensor.matmul(
                        out=zt[:, 6:8, :],
                        lhsT=ident[:],
                        rhs=ZX[d][:, 6:8, t * Bc : (t + 1) * Bc],
                        start=True,
                        stop=False,
                    )
                    for m in (6, 7):
                        for k in range(2):
                            rhs = (
                                hz[:, k, :]
                                if tprev is None
                                else H[d][:, k, tprev * Bc : (tprev + 1) * Bc]
                            )
                            nc.tensor.matmul(
                                out=zt[:, m, :],
                                lhsT=whh_sb[d][:, k, m * 128 : (m + 1) * 128],
                                rhs=rhs,
                                start=False,
                                stop=(m == 7 and k == 1),
                            )
                    nc.tensor.matmul(
                        out=zt[:, 0:6, :],
                        lhsT=ident[:],
                        rhs=ZX[d][:, 0:6, t * Bc : (t + 1) * Bc],
                        start=True,
                        stop=False,
                    )
                    for m in (0, 1, 2, 3, 4, 5):
                        for k in range(2):
                            rhs = (
                                hz[:, k, :]
                                if tprev is None
                                else H[d][:, k, tprev * Bc : (tprev + 1) * Bc]
                            )
                            nc.tensor.matmul(
                                out=zt[:, m, :],
                                lhsT=whh_sb[d][:, k, m * 128 : (m + 1) * 128],
                                rhs=rhs,
                                start=False,
                                stop=(m == 5 and k == 1),
                            )
                    # gate order (i, f, o, g); ACT reads PSUM directly
                    zf = gw.tile([128, 8, Bc], F32, tag=f"zf{d}")
                    nc.scalar.activation(out=zf[:, 6:8, :], in_=zt[:, 6:8, :], func=AF.Tanh)
                    nc.scalar.activation(out=zf[:, 0:6, :], in_=zt[:, 0:6, :], func=AF.Sigmoid)
                    a = gw2.tile([128, 2, Bc], F32, tag=f"a{d}")
                    nc.vector.tensor_mul(out=a[:], in0=zf[:, 2:4, :], in1=ct[d][:])
                    bb = gw2.tile([128, 2, Bc], F32, tag=f"b{d}")
                    nc.vector.tensor_mul(out=bb[:], in0=zf[:, 0:2, :], in1=zf[:, 6:8, :])
                    nc.vector.tensor_add(out=ct[d][:], in0=a[:], in1=bb[:])
                    tch = gw2.tile([128, 2, Bc], F32, tag=f"tc{d}")
                    nc.scalar.activation(out=tch[:], in_=ct[d][:], func=AF.Tanh)
                    nc.vector.tensor_mul(
                        out=H[d][:, :, t * Bc : (t + 1) * Bc],
                        in0=zf[:, 4:6, :],
                        in1=tch[:],
                    )

                for i in range(S):
                    lstm_step("b", S - 1 - i, None if i == 0 else S - i)
                    lstm_step("f", i, None if i == 0 else i - 1)

            # ---- emissions + numerator + CRF denominator
            if "e" not in phases:
                pass
            else:
             with ExitStack() as pen:
                nsing = pen.enter_context(tc.tile_pool(name="nsing", bufs=1))
                csing = pen.enter_context(tc.tile_pool(name="csing", bufs=1))

                with ExitStack() as pe1:
                    # emissions: emT [10, TOK] = w_out @ h^T + b_out (wide chunks)
                    ep = pe1.enter_context(tc.tile_pool(name="ep", bufs=2, space="PSUM"))
                    et = pe1.enter_context(tc.tile_pool(name="et", bufs=2, space="PSUM"))
                    for chk in range(NCHUNK):
                        ept = ep.tile([TAGSET, CW], F32, tag="ept")
                        for k4 in range(4):
                            dsrc = "f" if k4 < 2 else "b"
                            kk = k4 % 2
                            nc.tensor.matmul(
                                out=ept[:],
                                lhsT=wout_sb[:, k4, :],
                                rhs=H[dsrc][:, kk, chk * CW : (chk + 1) * CW],
                                start=(k4 == 0),
                                stop=(k4 == 3),
                            )
                        # raw emissions (bf16, for the numerator via transposes)
                        nc.scalar.activation(
                            out=emT[:, chk * CW : (chk + 1) * CW], in_=ept[:],
                            func=AF.Identity, bias=boutc_sb[:], scale=1.0,
                        )

                    # token-major emissions (numerator + batch-major rearrange)
                    for i in range(NTILE):
                        pt2 = et.tile([128, TAGSET], BF16, tag="pt2")
                        nc.tensor.transpose(
                            out=pt2[:], in_=emT[:, i * 128 : (i + 1) * 128],
                            identity=ident[0:TAGSET, 0:TAGSET],
                        )
                        nc.vector.tensor_copy(out=em_sb[:, i, :], in_=pt2[:])

                    # batch-major rearrange: partition p holds token t*Bc+b with
                    # b = p%Bc, r = p//Bc; tile i covers t = i*TPT + r
                    TPT = 128 // Bc
                    for r in range(TPT):
                        dst = emC[:, r, :]
                        dst3 = bass.AP(
                            tensor=dst.tensor,
                            offset=dst.offset,
                            ap=[dst.ap[0], [TPT * TAGSET, NTILE], dst.ap[1]],
                        )
                        nc.gpsimd.dma_start(
                            out=dst3, in_=em_sb[r * Bc : (r + 1) * Bc, :, :]
                        )
                    # exp over the whole batch-major emissions in one pass
                    nc.scalar.activation(out=eemC[:], in_=emC[:], func=AF.Exp)

                    # numerator: gold-path score via batched one-hot dot products
                    npool = pe1.enter_context(tc.tile_pool(name="npool", bufs=1))
                    junk = npool.tile([128, NTILE * NT * NT], F32)
                    junk2 = npool.tile([128, NTILE * NT], F32)
                    smat = npool.tile([128, 2 * NTILE], F32)
                    nc.vector.tensor_tensor(
                        out=_view3(junk2[:], [[NT, NTILE], [1, NT]]),
                        in0=em_sb[:, :, 1:TAGSET],
                        in1=_view3(ohem_sb[:], [[NT, NTILE], [1, NT]]),
                        op=ALU.mult,
                    )
                    nc.vector.reduce_sum(
                        out=smat[:, 0:NTILE],
                        in_=_view3(junk2[:], [[NT, NTILE], [1, NT]]),
                        axis=mybir.AxisListType.X,
                    )
                    nc.vector.tensor_tensor(
                        out=_view3(junk[:], [[NT * NT, NTILE], [1, NT * NT]]),
                        in0=_view3(ohtr_sb[:], [[NT * NT, NTILE], [1, NT * NT]]),
                        in1=_bcast_mid(trepPN_sb[:], NTILE),
                        op=ALU.mult,
                    )
                    nc.vector.reduce_sum(
                        out=smat[:, NTILE : 2 * NTILE],
                        in_=_view3(junk[:], [[NT * NT, NTILE], [1, NT * NT]]),
                        axis=mybir.AxisListType.X,
                    )
                    selp_pool = pe1.enter_context(
                        tc.tile_pool(name="selp", bufs=1, space="PSUM"))
                    selp = selp_pool.tile([Bc, 2 * NTILE], F32)
                    nc.tensor.matmul(
                        out=selp[:], lhsT=esel_sb[:], rhs=smat[:], start=True, stop=True
                    )
                    numm = nsing.tile([Bc, 1], F32)
                    nc.vector.reduce_sum(out=numm[:], in_=selp[:], axis=mybir.AxisListType.X)
                    stsc = nsing.tile([Bc, 1], F32)
                    jsmall = nsing.tile([Bc, NT], F32)
                    nc.vector.tensor_mul(out=jsmall[:], in0=ohst_sb[:], in1=strep_sb[:])
                    nc.vector.reduce_sum(out=stsc[:], in_=jsmall[:], axis=mybir.AxisListType.X)
                    ensc = nsing.tile([Bc, 1], F32)
                    nc.vector.tensor_mul(out=jsmall[:], in0=ohen_sb[:], in1=enrep_sb[:])
                    nc.vector.reduce_sum(out=ensc[:], in_=jsmall[:], axis=mybir.AxisListType.X)
                    nc.vector.tensor_add(out=ensc[:], in0=ensc[:], in1=stsc[:])
                    numt = nsing.tile([Bc, 1], F32)
                    nc.vector.tensor_add(out=numt[:], in0=numm[:], in1=ensc[:])

                # CRF denominator: linear-space forward, batch-major, all-DVE.
                # s_t[b, j] = sum_i s_{t-1}[b, i] * E2_{t-1}[b, j, i] where
                # E2_t[b, j, i] = eem[b, t, i] * exp(tr)[i, j] is precomputed
                # in chunks; the true alpha_t = s_t (*) eem_t is never formed.
                if "n" not in phases:
                    pass
                else:
                 with ExitStack() as pe2:
                    cp = pe2.enter_context(tc.tile_pool(name="cp", bufs=4))
                    e2p = pe2.enter_context(tc.tile_pool(name="e2p", bufs=2))
                    CH = 32  # E2 chunk length (timesteps)
                    cacc = csing.tile([Bc, 1], F32)
                    nc.vector.memset(cacc[:], 0.0)

                    s = cp.tile([Bc, NT], F32, tag="s")
                    nc.vector.tensor_copy(out=s[:], in_=estrep_sb[:])
                    E2 = None
                    for t in range(1, S):
                        tb = t - 1  # E2 index
                        if tb % CH == 0:
                            n = min(CH, (S - 1) - tb)
                            E2 = e2p.tile([Bc, CH * NT * NT], F32, tag="E2")
                            # E2[b, dt, j, i] = eem[b, tb+dt, i+1] * etrT[b, j, i]
                            eslice = eemC[:, tb, 1:TAGSET]  # [Bc, 9] at t=tb
                            nc.vector.tensor_tensor(
                                out=_view3(E2[:], [[NT * NT, n], [1, NT * NT]]),
                                in0=bass.AP(
                                    tensor=eslice.tensor, offset=eslice.offset,
                                    ap=[eslice.ap[0], [TAGSET, n], [0, NT], [1, NT]],
                                ),
                                in1=_bcast_mid(etrT_sb[:], n),
                                op=ALU.mult,
                            )
                        off = (tb % CH) * NT * NT
                        tmp = cp.tile([Bc, NT, NT], F32, tag="tmp")
                        nc.vector.tensor_tensor(
                            out=tmp[:], in0=_bcast_mid(s[:], NT),
                            in1=_view3(E2[:, off : off + NT * NT], [[NT, NT], [1, NT]]),
                            op=ALU.mult,
                        )
                        s_new = cp.tile([Bc, NT], F32, tag="s")
                        nc.vector.reduce_sum(
                            out=s_new[:], in_=tmp[:], axis=mybir.AxisListType.X
                        )
                        s = s_new
                        if t % RENORM == RENORM - 1 or t == S - 1:
                            cs = cp.tile([Bc, 1], F32, tag="cs")
                            nc.vector.reduce_sum(
                                out=cs[:], in_=s[:], axis=mybir.AxisListType.X
                            )
                            # ln(cs) + accumulate runs off the critical path (ACT)
                            lncs = cp.tile([Bc, 1], F32, tag="lncs")
                            nc.scalar.activation(out=lncs[:], in_=cs[:], func=AF.Ln)
                            nc.vector.tensor_add(out=cacc[:], in0=cacc[:], in1=lncs[:])
                            rec = cp.tile([Bc, 1], F32, tag="rec")
                            nc.vector.reciprocal(out=rec[:], in_=cs[:])
                            s_rn = cp.tile([Bc, NT], F32, tag="s")
                            nc.vector.tensor_tensor(
                                out=s_rn[:], in0=s[:], in1=_bcast_free(rec[:], NT),
                                op=ALU.mult,
                            )
                            s = s_rn

                    # den = cacc + ln(sum_j s[j] * eem[S-1, j] * eend[j])
                    a_last = csing.tile([Bc, NT], F32)
                    nc.vector.tensor_mul(
                        out=a_last[:], in0=s[:], in1=eemC[:, S - 1, 1:TAGSET]
                    )
                    af = csing.tile([Bc, NT], F32)
                    nc.vector.tensor_mul(out=af[:], in0=a_last[:], in1=eenrep_sb[:])
                    fs = csing.tile([Bc, 1], F32)
                    nc.vector.reduce_sum(out=fs[:], in_=af[:], axis=mybir.AxisListType.X)
                    lnf = csing.tile([Bc, 1], F32)
                    nc.scalar.activation(out=lnf[:], in_=fs[:], func=AF.Ln)
                    den = csing.tile([Bc, 1], F32)
                    nc.vector.tensor_add(out=den[:], in0=lnf[:], in1=cacc[:])

                    sc = csing.tile([Bc, 1], F32)
                    nc.vector.tensor_tensor(
                        out=sc[:], in0=numt[:], in1=den[:], op=ALU.subtract
                    )
                    nc.gpsimd.dma_start(out=scores_d[:], in_=sc[:])
                    dbg = csing.tile([Bc, 2], F32)
                    nc.vector.tensor_copy(out=dbg[:, 0:1], in_=numt[:])
                    nc.vector.tensor_copy(out=dbg[:, 1:2], in_=den[:])
                    nc.gpsimd.dma_start(out=dbg_d[:], in_=dbg[:])

        # hardware loops (tc.For_i) fail this walrus build's codegen
        # ("ISA wrong length"), so repeat is python-unrolled
        for _rep in range(repeat):
            body()

    _fixup_wait_limit(nc)
    return nc


# ---------------------------------------------------------------------------
# Host side
# ---------------------------------------------------------------------------

_PROGRAM_CACHE = {}
LAST_RESULTS = None


def _get_program(S, repeat=1):
    key = (S, repeat)
    if key not in _PROGRAM_CACHE:
        _PROGRAM_CACHE[key] = build_program(S, repeat=repeat)
    return _PROGRAM_CACHE[key]


def _tile_k(w, ktiles, cols):
    """[ktiles*128, cols] -> [128, ktiles, cols]"""
    return np.ascontiguousarray(w.reshape(ktiles, 128, cols).transpose(1, 0, 2))


def _prep_common(emb_table, w_ih_f, w_hh_f, b_ih_f, b_hh_f, w_ih_b, w_hh_b,
                 b_ih_b, b_hh_b, w_out, b_out, start_trans, end_trans, trans):
    f32 = np.float32
    com = {}
    com["emb"] = np.ascontiguousarray(emb_table, dtype=f32)
    gperm = np.concatenate([
        np.arange(0, 2 * Hd),            # i, f
        np.arange(3 * Hd, 4 * Hd),       # o
        np.arange(2 * Hd, 3 * Hd),       # g
    ])
    for d, wih, whh, bi, bh in (
        ("f", w_ih_f, w_hh_f, b_ih_f, b_hh_f),
        ("b", w_ih_b, w_hh_b, b_ih_b, b_hh_b),
    ):
        com[f"wih_{d}"] = _tile_k(wih[gperm].T.astype(nbf16), 2, 4 * Hd)
        com[f"whh_{d}"] = _tile_k(whh[gperm].T.astype(nbf16), 2, 4 * Hd)
        bias = (bi + bh).astype(f32)[gperm]
        com[f"bias_{d}"] = np.ascontiguousarray(bias.reshape(8, 128).T)
    com["wout"] = _tile_k(w_out.T.astype(nbf16), 4, TAGSET)
    com["boutc"] = np.ascontiguousarray(b_out.astype(f32)[:, None])
    com["etrT"] = np.tile(
        np.exp(trans.astype(f32)).T.reshape(1, -1), (Bc, 1))  # [Bc, j*9+i]
    com["estrep"] = np.tile(np.exp(start_trans.astype(f32))[None, :], (Bc, 1))
    com["eenrep"] = np.tile(np.exp(end_trans.astype(f32))[None, :], (Bc, 1))
    com["trepPN"] = np.tile(trans.astype(f32).reshape(1, -1), (128, 1))
    com["strep"] = np.tile(start_trans.astype(f32)[None, :], (Bc, 1))
    com["enrep"] = np.tile(end_trans.astype(f32)[None, :], (Bc, 1))
    com["esel"] = (
        (np.arange(128)[:, None] % Bc) == np.arange(Bc)[None, :]
    ).astype(f32)
    com["ident"] = np.eye(128, dtype=nbf16)
    return {k: np.ascontiguousarray(v) for k, v in com.items()}


def _prep_core(inputs, tags, c, S):
    f32 = np.float32
    NTILE = S * Bc // 128
    seqs = slice(c * Bc, (c + 1) * Bc)
    # token order tau = t*Bc + b
    idmat = np.asarray(inputs[seqs]).T.astype(np.int32)       # [S, Bc]
    ids_flat = idmat.reshape(-1)                               # [S*Bc]
    ids_col = np.ascontiguousarray(ids_flat.reshape(NTILE, 128).T)  # [128, NTILE]
    tags0 = np.asarray(tags[seqs]).T.astype(np.int64) - 1      # [S, Bc], 0..8
    eye9 = np.eye(NT, dtype=f32)
    eye81 = np.eye(NT * NT, dtype=f32)
    ohem = eye9[tags0.reshape(-1)]                             # [S*Bc, 9]
    ohem = ohem.reshape(NTILE, 128, NT).transpose(1, 0, 2)
    pair = tags0[:-1] * NT + tags0[1:]                         # [S-1, Bc]
    ohtr = np.zeros((S, Bc, NT * NT), dtype=f32)
    ohtr[1:] = eye81[pair]
    ohtr = ohtr.reshape(NTILE, 128, NT * NT).transpose(1, 0, 2)
    ohst = eye9[tags0[0]]                                      # [Bc, 9]
    ohen = eye9[tags0[-1]]
    return {
        "ids": ids_col,
        "ohem": np.ascontiguousarray(ohem.reshape(128, -1)),
        "ohtr": np.ascontiguousarray(ohtr.reshape(128, -1)),
        "ohst": ohst,
        "ohen": ohen,
    }


def _make_in_maps(inputs, tags, mask, emb_table, w_ih_f, w_hh_f, b_ih_f, b_hh_f,
                  w_ih_b, w_hh_b, b_ih_b, b_hh_b, w_out, b_out,
                  start_trans, end_trans, trans, S):
    com = _prep_common(
        np.asarray(emb_table), np.asarray(w_ih_f), np.asarray(w_hh_f),
        np.asarray(b_ih_f), np.asarray(b_hh_f), np.asarray(w_ih_b),
        np.asarray(w_hh_b), np.asarray(b_ih_b), np.asarray(b_hh_b),
        np.asarray(w_out), np.asarray(b_out), np.asarray(start_trans),
        np.asarray(end_trans), np.asarray(trans),
    )
    in_maps = []
    for c in range(NCORES):
        m = dict(com)
        m.update(_prep_core(np.asarray(inputs), np.asarray(tags), c, S))
        in_maps.append(m)
    return in_maps


def run(inputs, tags, mask, emb_table, w_ih_f, w_hh_f, b_ih_f, b_hh_f,
        w_ih_b, w_hh_b, b_ih_b, b_hh_b, w_out, b_out,
        start_trans, end_trans, trans, S=None, trace=False):
    global LAST_RESULTS
    inputs = np.asarray(inputs)
    if S is None:
        S = inputs.shape[1]
    nc = _get_program(S)
    in_maps = _make_in_maps(
        inputs, tags, mask, emb_table, w_ih_f, w_hh_f, b_ih_f, b_hh_f,
        w_ih_b, w_hh_b, b_ih_b, b_hh_b, w_out, b_out,
        start_trans, end_trans, trans, S)
    res = run_bass_kernel_spmd(
        nc, in_maps, core_ids=list(range(NCORES)), trace=trace
    )
    LAST_RESULTS = res
    scores = np.concatenate([r["scores"][:, 0] for r in res.results])
    loss = -np.mean(scores)
    return np.array(loss, dtype=np.float32)


def kernel(**inputs):
    return run(**inputs)


def make_timed_runner(S, in_maps, nc=None, repeat=1):
    """Build a persistent jitted sharded callable with device-resident inputs
    for timing repeated executions (axon has no NTFF hook in this container)."""
    import jax
    from jax.sharding import Mesh, PartitionSpec, NamedSharding
    from jax.experimental.shard_map import shard_map
    from concourse import bass2jax as b2j
    from concourse import mybir as _mybir

    if nc is None:
        nc = _get_program(S, repeat=repeat)
    b2j.install_neuronx_cc_hook()
    partition_name = nc.partition_id_tensor.name if nc.partition_id_tensor else None
    in_names, out_names, out_avals, zero_outs = [], [], [], []
    for alloc in nc.m.functions[0].allocations:
        if not isinstance(alloc, _mybir.MemoryLocationSet):
            continue
        name = alloc.memorylocations[0].name
        if alloc.kind == "ExternalInput":
            if name != partition_name:
                in_names.append(name)
        elif alloc.kind == "ExternalOutput":
            shape = tuple(alloc.tensor_shape)
            dtype = _mybir.dt.np(alloc.dtype)
            out_names.append(name)
            out_avals.append(jax.core.ShapedArray(shape, dtype))
            zero_outs.append(np.zeros(shape, dtype))
    n_params = len(in_names)
    all_in_names = list(in_names) + list(out_names)
    if partition_name is not None:
        all_in_names.append(partition_name)

    def _body(*args):
        operands = list(args)
        if partition_name is not None:
            operands.append(b2j.partition_id_tensor())
        outs = b2j._bass_exec_p.bind(
            *operands,
            out_avals=tuple(out_avals),
            in_names=tuple(all_in_names),
            out_names=tuple(out_names),
            lowering_input_output_aliases=(),
            sim_require_finite=True,
            sim_require_nnan=True,
            nc=nc,
        )
        return tuple(outs)

    n = len(in_maps)
    devices = jax.devices()[:n]
    mesh = Mesh(np.asarray(devices), ("core",))
    in_specs = (PartitionSpec("core"),) * (n_params + len(out_names))
    out_specs = (PartitionSpec("core"),) * len(out_names)
    sharded = jax.jit(
        shard_map(_body, mesh=mesh, in_specs=in_specs, out_specs=out_specs,
                  check_rep=False),
        keep_unused=True,
    )
    sh = NamedSharding(mesh, PartitionSpec("core"))
    concat_in = [
        jax.device_put(
            np.concatenate([np.asarray(in_maps[c][nm]) for c in range(n)], axis=0), sh
        )
        for nm in in_names
    ]
    concat_zeros = [
        jax.device_put(np.zeros((n * z.shape[0], *z.shape[1:]), z.dtype), sh)
        for z in zero_outs
    ]

    def call():
        outs = sharded(*concat_in, *concat_zeros)
        jax.block_until_ready(outs)
        return outs

    def call_async(n):
        """Dispatch n executions back-to-back; block only on the last.
        The axon client pipelines the execute RPCs, so the per-call
        tunnel latency amortizes across all n."""
        outs = None
        for _ in range(n):
            outs = sharded(*concat_in, *concat_zeros)
        jax.block_until_ready(outs)
        return outs

    call.out_names = out_names
    call.call_async = call_async
    return call
